# revision 5
# baseline (speedup 1.0000x reference)
"""Trainium2 Bass kernel for nn_DoubleLayer (e3nn-style double tensor-product layer).

Math per row b (layout x = [s(8) | v(8 vec channels, u-major xyz)]):
  layer(s, v; W) with irreps (ms x 0e + mv x 1o) -> (mw x 0e + mw x 1o):
    out_s[w]   = c0*(sum_uv s_u s_v Wss[u,v,w] + 1/sqrt3 * sum_uvi v_ui v_vi Wvv0[u,v,w])
    out_v[w,k] = c1*(1/sqrt3*(sum_uv s_u v_vk Wsv[u,v,w] + v_uk s_v Wvs[u,v,w])
                 + 1/sqrt6 * sum eps_ijk v_ui v_vj Wvv1[u,v,w])
  x -> tanh(s),v -> L1 -> si_norm -> tv_norm -> L2 -> si_norm -> sigmoid(s).

Kernel strategy v2 (pure data parallel over 8 cores, 32768 rows/core):
  For each 128-row chunk (batch rows on SBUF partitions):
    1. PE transpose z [128, nf] -> z^T (into a scratch region of the PSUM
       matmul tile); ACT copies it to SBUF (stationary for pass-1).
    2. PE pass-1: z^T @ bundle[nf, 1664] -> per-row intermediates M in PSUM.
       Bundle columns are host-packed weighted contractions; the cross-product
       +/- signs are baked into the bundle so no negated copies are needed.
       Layout: [0:512)   spart: (w, q) w-major, q = feature index
               [512:1664) vpart: (k, w, j) with j in [0,3mv):
                  j<mv: C (mult v[j,k]); mv<=j<2mv: D+ (mult v[u,(k+1)%3]);
                  j>=2mv: D- (mult v[u,(k+2)%3], sign baked into bundle).
    3. Fused custom-DVE MULSCAN ops: out = running prefix sum of
       (M * broadcast z) along the stream.  One op for spart, one per k for
       vpart.  Segment sums then drop out as boundary differences of the
       prefix stream (cheap strided tensor_sub), replacing the former
       separate product ops + 1-elem/cycle tensor_reduce passes.
  Norms (si_norm/tv_norm, per-row over channels) are batched across T chunks.
"""

import sys
import numpy as np

for _p in ("/opt/trn_rl_repo",):
    if _p not in sys.path:
        sys.path.append(_p)

MI, MH, MO = 8, 16, 8
NB = 262144
NCORES = 8
ROWS_PER_CORE = NB // NCORES
P = 128
T = 32                     # chunks per macro tile
MACRO = P * T              # 4096 rows
EPS_SI = 1e-9
EPS_TV = 1e-6
TINY = 1e-12

SP = 512                   # spart region width (mw * q_s) for both layers
VK = 384                   # per-k vpart region width (mw * 3mv) for both layers
MCOLS = SP + 3 * VK        # 1664 M columns per chunk
PMW = 1792                 # psum tile: 1664 M + 128 transpose scratch
PRODW = 1716               # prod tile: 1 + 512 + 3*(1+384) = 1668, padded for diff views


# ---------------------------------------------------------------------------
# custom DVE op: out = prefix_sum(in0 * in1) along the free stream
# ---------------------------------------------------------------------------

_MULSCAN = None


def _get_mulscan():
    global _MULSCAN
    if _MULSCAN is not None:
        return _MULSCAN
    from concourse import dve_ops
    from concourse.dve_spec import Spec, Src0, Src1, scan, AluOp, lower, _has_src1
    from concourse.dve_uop import DveOpSpec

    name = "MULSCAN_ANT"
    for op in dve_ops.OPS:
        if op.name == name:
            _MULSCAN = op
            return op

    def _ref(in0, in1, c0, c1, c2):
        b = (np.asarray(in0, np.float32) * np.asarray(in1, np.float32))
        p = b.shape[0]
        return np.cumsum(b.reshape(p, -1), axis=-1, dtype=np.float32).reshape(b.shape)

    spec = Spec(body=scan(AluOp.ADD, Src0 * Src1), reference=_ref)
    row = dve_ops._CUSTOM_DVE_ROW_BASE + len(dve_ops.OPS)
    shas = {}
    for ver in ("v3", "v4"):
        tmp = DveOpSpec(name=name, opcode=row, uops=lower(spec, ver=ver),
                        rd1_en=_has_src1(spec))
        shas[ver] = tmp.sha(ver)
    op = dve_ops.DveOp(name, spec, subdim=False, uops_sha=shas)
    dve_ops.OPS.append(op)
    dve_ops._SUB_OPCODE_FOR_NAME[name] = row
    dve_ops.CUSTOM_DVE_SPECS[name] = spec
    _MULSCAN = op
    return op


# ---------------------------------------------------------------------------
# bundle packing
# ---------------------------------------------------------------------------

def _build_bundle(ms, mv, mw, Wss, Wvv0, Wsv, Wvs, Wvv1):
    """Pack weighted-contraction bundle [nf, MCOLS], nf = ms + 3*mv.

    z feature layout: f in [0, ms) = s_f ; f = ms + 3*u + i = v[u, i].
    spart col (w, q) = w*q_s + q  (q_s = nf):
       q in [0, ms): rows s, val c0*Wss[q, f, w]
       q = ms+3u+i:  rows v[:, i], val c0/sqrt3*Wvv0[u, f, w]
    vpart col (k, w, j) = SP + k*mw*3mv + w*3mv + j:
       j in [0, mv):  C,  rows s,            val c1/sqrt3*(Wsv[f,j,w]+Wvs[j,f,w])
       j = mv + u:    D+, rows v[:,(k+2)%3], val +c1/sqrt6*Wvv1[u,f_v,w]
       j = 2mv + u:   D-, rows v[:,(k+1)%3], val -c1/sqrt6*Wvv1[u,f_v,w]
    """
    nf = ms + 3 * mv
    q_s = nf
    c0 = (ms * ms + mv * mv) ** -0.5
    c1 = (3.0 / (2 * ms * mv + mv * mv)) ** 0.5
    inv3 = 3.0 ** -0.5
    inv6 = 6.0 ** -0.5
    assert mw * q_s == SP and mw * 3 * mv == VK
    B = np.zeros((nf, MCOLS), np.float32)
    vrows = ms + 3 * np.arange(mv)
    for w in range(mw):
        for u in range(ms):
            B[0:ms, w * q_s + u] = c0 * Wss[u, :, w]
        for u in range(mv):
            for i in range(3):
                B[vrows + i, w * q_s + ms + 3 * u + i] = c0 * inv3 * Wvv0[u, :, w]
    for k in range(3):
        i1, i2 = (k + 1) % 3, (k + 2) % 3
        for w in range(mw):
            base = SP + k * VK + w * 3 * mv
            for m in range(mv):
                B[0:ms, base + m] = c1 * inv3 * (Wsv[:, m, w] + Wvs[m, :, w])
            for u in range(mv):
                B[vrows + i2, base + mv + u] = c1 * inv6 * Wvv1[u, :, w]
                B[vrows + i1, base + 2 * mv + u] = -c1 * inv6 * Wvv1[u, :, w]
    return B


# ---------------------------------------------------------------------------
# numpy emulation of the device pipeline (for bundle/layout validation)
# ---------------------------------------------------------------------------

def _np_layer(z, bundle, ms, mv, mw):
    """Emulates MM1 + scan/diff segment sums.  yv returned K-MAJOR [n, k, w]."""
    nf = ms + 3 * mv
    n = z.shape[0]
    M = z @ bundle  # [n, MCOLS]
    v = z[:, ms:nf].reshape(n, mv, 3)
    # spart: products z_q * M[w, q], summed over q
    sM = M[:, :SP].reshape(n, mw, nf)
    out_s = np.einsum('nq,nwq->nw', z, sM)
    # vpart per k: multiplier groups [v[:,k] | v[:,(k+1)%3] | v[:,(k+2)%3]]
    out_v = np.zeros((n, 3, mw), np.float32)
    for k in range(3):
        mult = np.concatenate([v[:, :, k], v[:, :, (k + 1) % 3],
                               v[:, :, (k + 2) % 3]], axis=1)  # [n, 3mv]
        Mk = M[:, SP + k * VK: SP + (k + 1) * VK].reshape(n, mw, 3 * mv)
        out_v[:, k, :] = np.einsum('nj,nwj->nw', mult, Mk)
    return out_s, out_v  # out_v k-major


def _np_si_norm(ys, yv):
    """yv k-major [n, 3, m]."""
    n, mh = ys.shape
    std_s = ys.std(axis=1, ddof=1)
    ys = ys / (std_s[:, None] + EPS_SI)
    norm1 = np.sqrt((yv ** 2).sum(axis=1) + EPS_SI)   # [n, m]
    std_v = norm1.std(axis=1, ddof=1)
    yv = yv / (std_v[:, None, None] + EPS_SI)
    return ys, yv


def _np_tv_norm(xs, xv):
    """xv k-major [n, 3, m]."""
    xs = xs / np.sqrt((xs ** 2).sum(axis=1, keepdims=True) + EPS_TV)
    norm1 = np.sqrt((xv ** 2).sum(axis=2) + EPS_TV)   # [n, 3]
    nm = norm1.mean(axis=1)
    xv = xv / (nm[:, None, None] + EPS_TV)
    return xs, xv


def reference_numpy(x, B1, B2):
    xs = np.tanh(x[:, :MI])
    z1 = np.concatenate([xs, x[:, MI:]], axis=1)
    ys, yv = _np_layer(z1, B1, MI, MI, MH)
    ys, yv = _np_si_norm(ys, yv)
    ys, yv = _np_tv_norm(ys, yv)
    # z2 features: [s | v u-major xyz] -> from k-major yv
    z2 = np.concatenate([ys, yv.transpose(0, 2, 1).reshape(x.shape[0], -1)], axis=1)
    zs, zv = _np_layer(z2, B2, MH, MH, MO)
    zs, zv = _np_si_norm(zs, zv)
    zs = 1.0 / (1.0 + np.exp(-zs))
    return np.concatenate([zs, zv.transpose(0, 2, 1).reshape(x.shape[0], -1)],
                          axis=1).astype(np.float32)


# ---------------------------------------------------------------------------
# device kernel
# ---------------------------------------------------------------------------

_PROGRAM_CACHE = {}


def _emit_layer_chunk(nc, psum, prod_pool, vtx_pool, t, xm_view, nf_stride,
                      bundle_sb, ztall, npack, ms, mv, mw, ys_macro, yv_macro,
                      ident, tag):
    """One 128-row chunk of one tensor-product layer.

    xm_view: [128, T*nf_stride] SBUF macro tile holding z per chunk
    ztall:   [128, (T//npack)*P] SBUF tile of transposed z (npack chunks/tile)
    ys_macro: [128, T*mw]; yv_macro: [128, T*3mw] (k-major per chunk)
    """
    import concourse.mybir as mybir
    f32 = mybir.dt.float32
    AF = mybir.ActivationFunctionType
    nf = ms + 3 * mv
    MULSCAN = _get_mulscan()

    pm = psum.tile([P, PMW], f32, tag="pm")
    g = t // npack
    if t % npack == 0:
        nc.tensor.transpose(pm[:, MCOLS:MCOLS + P], xm_view[:, g * P:(g + 1) * P],
                            ident)
        nc.scalar.copy(ztall[:, g * P:(g + 1) * P], pm[:, MCOLS:MCOLS + P])
    r0 = nf * (t % npack)
    lhsT = ztall[r0:r0 + nf, g * P:(g + 1) * P]
    for n0, n1 in ((0, 512), (512, 1024), (1024, 1536), (1536, MCOLS)):
        nc.tensor.matmul(pm[:, n0:n1], lhsT, bundle_sb[r0:r0 + nf, n0:n1],
                         start=True, stop=True, tile_position=(r0, 0))

    z_sl = xm_view[:, t * nf_stride: t * nf_stride + nf]
    # vTx: [P, 5*mv] = v transposed to (i, u) layout, i wrapped 0,1,2,0,1
    vtx = vtx_pool.tile([P, 5 * mv], f32, tag=f"vtx{tag}")
    v_iu = z_sl[:, ms:nf].rearrange("p (u i) -> p i u", i=3)
    nc.vector.tensor_copy(vtx[:, 0:3 * mv].rearrange("p (i u) -> p i u", u=mv), v_iu)
    nc.vector.tensor_copy(vtx[:, 3 * mv:5 * mv].rearrange("p (i u) -> p i u", u=mv),
                          v_iu[:, 0:2, :])

    prod = prod_pool.tile([P, PRODW], f32, tag=f"prod{tag}")
    # spart scan: stream [w, q], prefix written at cols [1, 513)
    nc.vector._custom_dve(
        MULSCAN,
        out=prod[:, 1:1 + SP].rearrange("p (w q) -> p w q", q=nf),
        in0=pm[:, 0:SP].rearrange("p (w q) -> p w q", q=nf),
        in1=z_sl.unsqueeze(1).broadcast_to([P, mw, nf]),
    )
    # vpart scans: per k, stream [w, j], prefix at cols [VB+k*385+1, +384)
    VB = 1 + SP
    for k in range(3):
        nc.vector._custom_dve(
            MULSCAN,
            out=prod[:, VB + k * 385 + 1: VB + k * 385 + 1 + VK].rearrange(
                "p (w j) -> p w j", j=3 * mv),
            in0=pm[:, SP + k * VK: SP + (k + 1) * VK].rearrange(
                "p (w j) -> p w j", j=3 * mv),
            in1=vtx[:, k * mv:(k + 3) * mv].unsqueeze(1).broadcast_to(
                [P, mw, 3 * mv]),
        )
    # boundary diffs -> segment sums
    q_s = nf
    ends_s = prod[:, q_s:q_s + SP].rearrange("p (w q) -> p w q", q=q_s)[:, :, 0]
    starts_s = prod[:, 0:SP].rearrange("p (w q) -> p w q", q=q_s)[:, :, 0]
    nc.vector.tensor_sub(ys_macro[:, t * mw:(t + 1) * mw], ends_s, starts_s)
    j3 = 3 * mv
    ends_v = prod[:, VB + j3:VB + j3 + 3 * 385].rearrange(
        "p (k r) -> p k r", r=385)[:, :, 0:VK].rearrange(
        "p k (w j) -> p k w j", j=j3)[:, :, :, 0]
    starts_v = prod[:, VB:VB + 3 * 385].rearrange(
        "p (k r) -> p k r", r=385)[:, :, 0:VK].rearrange(
        "p k (w j) -> p k w j", j=j3)[:, :, :, 0]
    nc.vector.tensor_sub(
        yv_macro[:, t * 3 * mw:(t + 1) * 3 * mw].rearrange(
            "p (k w) -> p k w", w=mw),
        ends_v, starts_v)


def _emit_program(nc, tc, x_d, b1_d, b2_d, out_d, rows, repeat=1):
    import concourse.mybir as mybir
    from concourse.masks import make_identity
    AF = mybir.ActivationFunctionType
    ALU = mybir.AluOpType
    AX = mybir.AxisListType
    f32 = mybir.dt.float32
    from contextlib import ExitStack

    nmacro = rows // MACRO
    ctx = ExitStack()
    with ctx:
        cpool = ctx.enter_context(tc.tile_pool(name="const", bufs=1))
        ident = cpool.tile([P, P], f32)
        make_identity(nc, ident[:])

        def _bias_tile(val, idx=[0]):
            bt = cpool.tile([P, 1], f32, tag=f"bias{idx[0]}")
            idx[0] += 1
            nc.gpsimd.memset(bt[:], float(val))
            return bt[:]

        b_tiny = _bias_tile(TINY)
        b_si = _bias_tile(EPS_SI)
        b_tv = _bias_tile(EPS_TV)
        b_v1 = _bias_tile(MH * EPS_SI / (MH - 1) + TINY)
        b_v2 = _bias_tile(MO * EPS_SI / (MO - 1) + TINY)
        b1_sb = cpool.tile([P, MCOLS], f32)
        for r in range(4):
            nc.sync.dma_start(out=b1_sb[32 * r:32 * (r + 1), :], in_=b1_d[:])
        b2_sb = cpool.tile([P, MCOLS], f32)
        for r in range(2):
            nc.sync.dma_start(out=b2_sb[64 * r:64 * (r + 1), :], in_=b2_d[:])

        io_pool = ctx.enter_context(tc.tile_pool(name="io", bufs=2))
        mid_pool = ctx.enter_context(tc.tile_pool(name="mid", bufs=2))
        nrm_pool = ctx.enter_context(tc.tile_pool(name="nrm", bufs=2))
        zt_pool = ctx.enter_context(tc.tile_pool(name="zt", bufs=2))
        prod_pool = ctx.enter_context(tc.tile_pool(name="prod", bufs=2))
        vtx_pool = ctx.enter_context(tc.tile_pool(name="vtx", bufs=2))
        psum = ctx.enter_context(tc.tile_pool(name="psum", bufs=2, space="PSUM"))

        # pre-zero the pad columns of all prod buffers (scans never write
        # them; the diff ops read them as the "prefix before first segment")
        for tag in ("1", "2"):
            for _ in range(2):
                pz = prod_pool.tile([P, PRODW], f32, tag=f"prod{tag}")
                nc.gpsimd.memset(pz[:], 0.0)

        for m in list(range(nmacro)) * repeat:
            xm = io_pool.tile([P, T * 32], f32, tag="xm")
            xview = x_d[m * MACRO:(m + 1) * MACRO, :].rearrange("(t p) f -> p t f", p=P)
            nc.sync.dma_start(out=xm[:].rearrange("p (t f) -> p t f", f=32), in_=xview)
            # tanh on scalar features (in place)
            xs_view = xm[:].rearrange("p (t f) -> p t f", f=32)[:, :, 0:MI]
            nc.scalar.activation(xs_view, xs_view, AF.Tanh)

            ztall1 = zt_pool.tile([P, (T // 4) * P], f32, tag="ztall1")
            ys8 = nrm_pool.tile([P, T * MH], f32, tag="ys8")
            yv8 = nrm_pool.tile([P, T * 3 * MH], f32, tag="yv8")
            for t in range(T):
                _emit_layer_chunk(nc, psum, prod_pool, vtx_pool, t, xm[:], 32,
                                  b1_sb[:], ztall1[:], 4, MI, MI, MH,
                                  ys8[:], yv8[:], ident[:], "1")

            # ---- si_norm(1) + tv_norm, batched over T chunks ----
            # yv8 layout per chunk: (k, u) k-major
            mh = MH
            ys8v = ys8[:].rearrange("p (t f) -> p t f", f=mh)
            yv8v = yv8[:].rearrange("p (t f) -> p t f", f=3 * mh)
            sq_s = nrm_pool.tile([P, T * mh], f32, tag="sq_s")
            nc.vector.tensor_mul(sq_s[:], ys8[:], ys8[:])
            sumsq_s = nrm_pool.tile([P, T], f32, tag="n1a")
            nc.vector.tensor_reduce(sumsq_s[:], sq_s[:].rearrange("p (t f) -> p t f", f=mh),
                                    axis=AX.X, op=ALU.add)
            sum_s = nrm_pool.tile([P, T], f32, tag="n1b")
            nc.vector.tensor_reduce(sum_s[:], ys8v, axis=AX.X, op=ALU.add)
            s2 = nrm_pool.tile([P, T], f32, tag="n1c")
            nc.scalar.activation(s2[:], sum_s[:], AF.Square, scale=float(mh) ** -0.5)
            varnum = nrm_pool.tile([P, T], f32, tag="n1d")
            nc.vector.tensor_sub(varnum[:], sumsq_s[:], s2[:])
            std_s = nrm_pool.tile([P, T], f32, tag="n1e")
            nc.scalar.activation(std_s[:], varnum[:], AF.Sqrt,
                                 scale=1.0 / (mh - 1), bias=b_tiny)
            stde_s = nrm_pool.tile([P, T], f32, tag="n1f")
            nc.vector.tensor_scalar_add(stde_s[:], std_s[:], EPS_SI)
            inv_s = nrm_pool.tile([P, T], f32, tag="n1g")
            nc.vector.reciprocal(inv_s[:], stde_s[:])

            sq_v = nrm_pool.tile([P, T * 3 * mh], f32, tag="sq_v")
            nc.vector.tensor_mul(sq_v[:], yv8[:], yv8[:])
            # n2u[t, u] = sum_k sq_v[t, k, u]  (k-major: k stride mh, u stride 1)
            n2u = nrm_pool.tile([P, T * mh], f32, tag="n2u")
            nc.vector.tensor_reduce(
                n2u[:].rearrange("p (t u) -> p t u", u=mh),
                sq_v[:].rearrange("p (t k u) -> p t u k", k=3, u=mh),
                axis=AX.X, op=ALU.add)
            norm1 = nrm_pool.tile([P, T * mh], f32, tag="norm1")
            nc.scalar.activation(norm1[:], n2u[:], AF.Sqrt, bias=b_si)
            rn = nrm_pool.tile([P, T], f32, tag="n1h")
            nc.vector.tensor_reduce(rn[:], n2u[:].rearrange("p (t u) -> p t u", u=mh),
                                    axis=AX.X, op=ALU.add)
            sum_n = nrm_pool.tile([P, T], f32, tag="n1i")
            nc.vector.tensor_reduce(sum_n[:], norm1[:].rearrange("p (t u) -> p t u", u=mh),
                                    axis=AX.X, op=ALU.add)
            s2n = nrm_pool.tile([P, T], f32, tag="n1j")
            nc.scalar.activation(s2n[:], sum_n[:], AF.Square, scale=float(mh) ** -0.5)
            varn = nrm_pool.tile([P, T], f32, tag="n1k")
            nc.vector.tensor_sub(varn[:], rn[:], s2n[:])
            std_v = nrm_pool.tile([P, T], f32, tag="n1l")
            nc.scalar.activation(std_v[:], varn[:], AF.Sqrt, scale=1.0 / (mh - 1),
                                 bias=b_v1)
            stde_v = nrm_pool.tile([P, T], f32, tag="n1m")
            nc.vector.tensor_scalar_add(stde_v[:], std_v[:], EPS_SI)
            inv_v = nrm_pool.tile([P, T], f32, tag="n1n")
            nc.vector.reciprocal(inv_v[:], stde_v[:])

            # tv_norm scalars
            invs2 = nrm_pool.tile([P, T], f32, tag="n1o")
            nc.vector.tensor_mul(invs2[:], inv_s[:], inv_s[:])
            q_sA = nrm_pool.tile([P, T], f32, tag="n1p")
            nc.vector.tensor_mul(q_sA[:], sumsq_s[:], invs2[:])
            rt_s = nrm_pool.tile([P, T], f32, tag="n1q")
            nc.scalar.activation(rt_s[:], q_sA[:], AF.Sqrt, bias=b_tv)
            invt_s = nrm_pool.tile([P, T], f32, tag="n1r")
            nc.vector.reciprocal(invt_s[:], rt_s[:])
            scale_s = nrm_pool.tile([P, T], f32, tag="n1s")
            nc.vector.tensor_mul(scale_s[:], inv_s[:], invt_s[:])

            # ni_raw[t, k] = sum_u sq_v[t, k, u]  (k-major: u innermost)
            ni_raw = nrm_pool.tile([P, T * 3], f32, tag="n1t")
            nc.vector.tensor_reduce(
                ni_raw[:].rearrange("p (t i) -> p t i", i=3),
                sq_v[:].rearrange("p (t k u) -> p t k u", k=3, u=mh),
                axis=AX.X, op=ALU.add)
            iv2 = nrm_pool.tile([P, T], f32, tag="n1u")
            nc.vector.tensor_mul(iv2[:], inv_v[:], inv_v[:])
            ni_tv = nrm_pool.tile([P, T * 3], f32, tag="n1v")
            nc.vector.tensor_mul(
                ni_tv[:].rearrange("p (t i) -> p t i", i=3),
                ni_raw[:].rearrange("p (t i) -> p t i", i=3),
                iv2[:].unsqueeze(2).broadcast_to([P, T, 3]))
            n1tv = nrm_pool.tile([P, T * 3], f32, tag="n1w")
            nc.scalar.activation(n1tv[:], ni_tv[:], AF.Sqrt, bias=b_tv)
            nm3 = nrm_pool.tile([P, T], f32, tag="n1x")
            nc.vector.tensor_reduce(nm3[:], n1tv[:].rearrange("p (t i) -> p t i", i=3),
                                    axis=AX.X, op=ALU.add)
            nme = nrm_pool.tile([P, T], f32, tag="n1y")
            nc.vector.tensor_scalar(nme[:], nm3[:], 1.0 / 3.0, EPS_TV,
                                    op0=ALU.mult, op1=ALU.add)
            invtv = nrm_pool.tile([P, T], f32, tag="n1z")
            nc.vector.reciprocal(invtv[:], nme[:])
            scale_v = nrm_pool.tile([P, T], f32, tag="n1A")
            nc.vector.tensor_mul(scale_v[:], inv_v[:], invtv[:])

            # apply scales -> zmid [128, T*64] (features [s(16) | v u-major xyz])
            zmid = mid_pool.tile([P, T * 64], f32, tag="zmid")
            zmv = zmid[:].rearrange("p (t f) -> p t f", f=64)
            nc.vector.tensor_mul(
                zmv[:, :, 0:MH], ys8v,
                scale_s[:].unsqueeze(2).broadcast_to([P, T, MH]))
            # v: out (u, k) u-major <- yv8 (k, u) k-major, times scale_v
            nc.vector.tensor_mul(
                zmv[:, :, MH:64].rearrange("p t (u k) -> p t u k", k=3, u=mh),
                yv8v.rearrange("p t (k u) -> p t u k", k=3, u=mh),
                scale_v[:].unsqueeze(2).unsqueeze(3).broadcast_to([P, T, mh, 3]))

            # ---- layer 2 ----
            ztall2 = zt_pool.tile([P, (T // 2) * P], f32, tag="ztall2")
            zs8 = nrm_pool.tile([P, T * MO], f32, tag="zs8")
            zv8 = nrm_pool.tile([P, T * 3 * MO], f32, tag="zv8")
            for t in range(T):
                _emit_layer_chunk(nc, psum, prod_pool, vtx_pool, t, zmid[:], 64,
                                  b2_sb[:], ztall2[:], 2, MH, MH, MO,
                                  zs8[:], zv8[:], ident[:], "2")

            # ---- si_norm(2) ----
            mo = MO
            zs8v = zs8[:].rearrange("p (t f) -> p t f", f=mo)
            zv8v = zv8[:].rearrange("p (t f) -> p t f", f=3 * mo)
            sq_s2 = nrm_pool.tile([P, T * mo], f32, tag="sq_s2")
            nc.vector.tensor_mul(sq_s2[:], zs8[:], zs8[:])
            sumsq2 = nrm_pool.tile([P, T], f32, tag="n2a")
            nc.vector.tensor_reduce(sumsq2[:], sq_s2[:].rearrange("p (t f) -> p t f", f=mo),
                                    axis=AX.X, op=ALU.add)
            sum2 = nrm_pool.tile([P, T], f32, tag="n2b")
            nc.vector.tensor_reduce(sum2[:], zs8v, axis=AX.X, op=ALU.add)
            s22 = nrm_pool.tile([P, T], f32, tag="n2c")
            nc.scalar.activation(s22[:], sum2[:], AF.Square, scale=float(mo) ** -0.5)
            varnum2 = nrm_pool.tile([P, T], f32, tag="n2d")
            nc.vector.tensor_sub(varnum2[:], sumsq2[:], s22[:])
            std_s2 = nrm_pool.tile([P, T], f32, tag="n2e")
            nc.scalar.activation(std_s2[:], varnum2[:], AF.Sqrt,
                                 scale=1.0 / (mo - 1), bias=b_tiny)
            stde_s2 = nrm_pool.tile([P, T], f32, tag="n2f")
            nc.vector.tensor_scalar_add(stde_s2[:], std_s2[:], EPS_SI)
            inv_s2 = nrm_pool.tile([P, T], f32, tag="n2g")
            nc.vector.reciprocal(inv_s2[:], stde_s2[:])

            sq_v2 = nrm_pool.tile([P, T * 3 * mo], f32, tag="sq_v2")
            nc.vector.tensor_mul(sq_v2[:], zv8[:], zv8[:])
            n2u2 = nrm_pool.tile([P, T * mo], f32, tag="n2u2")
            nc.vector.tensor_reduce(
                n2u2[:].rearrange("p (t u) -> p t u", u=mo),
                sq_v2[:].rearrange("p (t k u) -> p t u k", k=3, u=mo),
                axis=AX.X, op=ALU.add)
            norm12 = nrm_pool.tile([P, T * mo], f32, tag="norm12")
            nc.scalar.activation(norm12[:], n2u2[:], AF.Sqrt, bias=b_si)
            rn2 = nrm_pool.tile([P, T], f32, tag="n2h")
            nc.vector.tensor_reduce(rn2[:], n2u2[:].rearrange("p (t u) -> p t u", u=mo),
                                    axis=AX.X, op=ALU.add)
            sum_n2 = nrm_pool.tile([P, T], f32, tag="n2i")
            nc.vector.tensor_reduce(sum_n2[:], norm12[:].rearrange("p (t u) -> p t u", u=mo),
                                    axis=AX.X, op=ALU.add)
            s2n2 = nrm_pool.tile([P, T], f32, tag="n2j")
            nc.scalar.activation(s2n2[:], sum_n2[:], AF.Square, scale=float(mo) ** -0.5)
            varn2 = nrm_pool.tile([P, T], f32, tag="n2k")
            nc.vector.tensor_sub(varn2[:], rn2[:], s2n2[:])
            std_v2 = nrm_pool.tile([P, T], f32, tag="n2l")
            nc.scalar.activation(std_v2[:], varn2[:], AF.Sqrt, scale=1.0 / (mo - 1),
                                 bias=b_v2)
            stde_v2 = nrm_pool.tile([P, T], f32, tag="n2m")
            nc.vector.tensor_scalar_add(stde_v2[:], std_v2[:], EPS_SI)
            inv_v2 = nrm_pool.tile([P, T], f32, tag="n2n")
            nc.vector.reciprocal(inv_v2[:], stde_v2[:])

            # scale, sigmoid, assemble output macro [128, T*32]
            outm = io_pool.tile([P, T * 32], f32, tag="outm")
            outv = outm[:].rearrange("p (t f) -> p t f", f=32)
            tmp_s = nrm_pool.tile([P, T * mo], f32, tag="tmp_s")
            nc.vector.tensor_mul(
                tmp_s[:].rearrange("p (t f) -> p t f", f=mo), zs8v,
                inv_s2[:].unsqueeze(2).broadcast_to([P, T, mo]))
            nc.scalar.activation(outv[:, :, 0:MO],
                                 tmp_s[:].rearrange("p (t f) -> p t f", f=mo),
                                 AF.Sigmoid)
            # v out (w, k) w-major <- zv8 (k, w) k-major
            nc.vector.tensor_mul(
                outv[:, :, MO:32].rearrange("p t (w k) -> p t w k", k=3, w=mo),
                zv8v.rearrange("p t (k w) -> p t w k", k=3, w=mo),
                inv_v2[:].unsqueeze(2).unsqueeze(3).broadcast_to([P, T, mo, 3]))

            oview = out_d[m * MACRO:(m + 1) * MACRO, :].rearrange("(t p) f -> p t f", p=P)
            nc.sync.dma_start(out=oview, in_=outm[:].rearrange("p (t f) -> p t f", f=32))


def _build_program(rows, repeat=1):
    import concourse.bacc as bacc
    import concourse.tile as tile
    import concourse.mybir as mybir
    f32 = mybir.dt.float32

    _get_mulscan()
    nc = bacc.Bacc("TRN2", target_bir_lowering=False, debug=False,
                   enable_asserts=False, num_devices=NCORES)
    x_d = nc.dram_tensor("x", [rows, 32], f32, kind="ExternalInput").ap()
    b1_d = nc.dram_tensor("b1", [32, MCOLS], f32, kind="ExternalInput").ap()
    b2_d = nc.dram_tensor("b2", [64, MCOLS], f32, kind="ExternalInput").ap()
    out_d = nc.dram_tensor("out", [rows, 32], f32, kind="ExternalOutput").ap()

    with tile.TileContext(nc) as tc:
        _emit_program(nc, tc, x_d, b1_d, b2_d, out_d, rows, repeat)
    nc.compile()
    return nc


def _get_program(rows, repeat=1):
    key = (rows, repeat)
    if key not in _PROGRAM_CACHE:
        _PROGRAM_CACHE[key] = _build_program(rows, repeat)
    return _PROGRAM_CACHE[key]


_RUN_CACHE = {}


def _get_runner(rows, repeat):
    """Build (once) a cached jitted shard_map executable for the program."""
    key = (rows, repeat)
    if key in _RUN_CACHE:
        return _RUN_CACHE[key]
    import jax
    import numpy as _np
    from jax.sharding import Mesh, PartitionSpec
    try:
        from jax.experimental.shard_map import shard_map
    except Exception:
        from jax.shard_map import shard_map  # newer jax
    from concourse import bass2jax
    import concourse.mybir as mybir

    nc = _get_program(rows, repeat)
    bass2jax.install_neuronx_cc_hook()
    partition_name = nc.partition_id_tensor.name if nc.partition_id_tensor else None
    in_names, out_names, out_avals, zero_outs = [], [], [], []
    for alloc in nc.m.functions[0].allocations:
        if not isinstance(alloc, mybir.MemoryLocationSet):
            continue
        name = alloc.memorylocations[0].name
        if alloc.kind == "ExternalInput":
            if name != partition_name:
                in_names.append(name)
        elif alloc.kind == "ExternalOutput":
            shape = tuple(alloc.tensor_shape)
            dtype = mybir.dt.np(alloc.dtype)
            out_names.append(name)
            out_avals.append(jax.core.ShapedArray(shape, dtype))
            zero_outs.append(_np.zeros(shape, dtype))
    n_params = len(in_names)
    n_outs = len(out_avals)
    all_in_names = list(in_names) + list(out_names)
    if partition_name is not None:
        all_in_names.append(partition_name)
    donate = tuple(range(n_params, n_params + n_outs))

    def _body(*args):
        operands = list(args)
        if partition_name is not None:
            operands.append(bass2jax.partition_id_tensor())
        outs = bass2jax._bass_exec_p.bind(
            *operands,
            out_avals=tuple(out_avals),
            in_names=tuple(all_in_names),
            out_names=tuple(out_names),
            lowering_input_output_aliases=(),
            sim_require_finite=True,
            sim_require_nnan=True,
            nc=nc,
        )
        return tuple(outs)

    devices = jax.devices()[:NCORES]
    mesh = Mesh(_np.asarray(devices), ("core",))
    in_specs = (PartitionSpec("core"),) * (n_params + n_outs)
    out_specs = (PartitionSpec("core"),) * n_outs
    sharded = jax.jit(
        shard_map(_body, mesh=mesh, in_specs=in_specs, out_specs=out_specs,
                  check_rep=False),
        donate_argnums=donate, keep_unused=True,
    )
    runner = (sharded, in_names, out_names, out_avals, zero_outs)
    _RUN_CACHE[key] = runner
    return runner


_ZERO_CACHE = {}


def _run_cached(rows, repeat, full_inputs):
    """full_inputs: dict name -> already-concatenated (NCORES*rows0, ...) array."""
    import numpy as _np
    sharded, in_names, out_names, out_avals, zero_outs = _get_runner(rows, repeat)
    concat_in = [full_inputs[nm] for nm in in_names]
    key = (rows, repeat)
    if key not in _ZERO_CACHE:
        _ZERO_CACHE[key] = [
            _np.zeros((NCORES * z.shape[0], *z.shape[1:]), z.dtype)
            for z in zero_outs]
    out_arrs = sharded(*concat_in, *_ZERO_CACHE[key])
    i = out_names.index("out")
    return _np.asarray(out_arrs[i]).reshape(NCORES, *out_avals[i].shape)


def kernel(x, w1_ss, w1_vv0, w1_sv, w1_vs, w1_vv1,
           w2_ss, w2_vv0, w2_sv, w2_vs, w2_vv1, _trace=False, _repeat=1):
    from concourse import bass_utils

    x = np.asarray(x, dtype=np.float32)
    B1 = _build_bundle(MI, MI, MH, np.asarray(w1_ss), np.asarray(w1_vv0),
                       np.asarray(w1_sv), np.asarray(w1_vs), np.asarray(w1_vv1))
    B2 = _build_bundle(MH, MH, MO, np.asarray(w2_ss), np.asarray(w2_vv0),
                       np.asarray(w2_sv), np.asarray(w2_vs), np.asarray(w2_vv1))

    rows = x.shape[0] // NCORES
    if _trace:
        shards = x.reshape(NCORES, rows, 32)
        in_maps = [{"x": np.ascontiguousarray(shards[i]), "b1": B1, "b2": B2}
                   for i in range(NCORES)]
        nc = _get_program(rows, _repeat)
        res = bass_utils.run_bass_kernel_spmd(nc, in_maps,
                                              core_ids=list(range(NCORES)),
                                              trace=True)
        out = np.concatenate([res.results[i]["out"] for i in range(NCORES)], axis=0)
        return out, res
    full_inputs = {"x": np.ascontiguousarray(x),
                   "b1": np.tile(B1, (NCORES, 1)),
                   "b2": np.tile(B2, (NCORES, 1))}
    try:
        per_core = _run_cached(rows, _repeat, full_inputs)
        return np.ascontiguousarray(per_core.reshape(rows * NCORES, 32))
    except Exception:
        shards = x.reshape(NCORES, rows, 32)
        in_maps = [{"x": np.ascontiguousarray(shards[i]), "b1": B1, "b2": B2}
                   for i in range(NCORES)]
        nc = _get_program(rows, _repeat)
        res = bass_utils.run_bass_kernel_spmd(nc, in_maps,
                                              core_ids=list(range(NCORES)))
        return np.concatenate([res.results[i]["out"] for i in range(NCORES)], axis=0)


# revision 10
# speedup vs baseline: 1.3760x; 1.3760x over previous
"""Trainium2 Bass kernel for nn_DoubleLayer (e3nn-style double tensor-product layer).

Math per row b (layout x = [s(8) | v(8 vec channels, u-major xyz)]):
  layer(s, v; W) with irreps (ms x 0e + mv x 1o) -> (mw x 0e + mw x 1o):
    out_s[w]   = c0*(sum_uv s_u s_v Wss[u,v,w] + 1/sqrt3 * sum_uvi v_ui v_vi Wvv0[u,v,w])
    out_v[w,k] = c1*(1/sqrt3*(sum_uv s_u v_vk Wsv[u,v,w] + v_uk s_v Wvs[u,v,w])
                 + 1/sqrt6 * sum eps_ijk v_ui v_vj Wvv1[u,v,w])
  x -> tanh(s),v -> L1 -> si_norm -> tv_norm -> L2 -> si_norm -> sigmoid(s).

Kernel strategy v2 (pure data parallel over 8 cores, 32768 rows/core):
  For each 128-row chunk (batch rows on SBUF partitions):
    1. PE transpose z [128, nf] -> z^T (into a scratch region of the PSUM
       matmul tile); ACT copies it to SBUF (stationary for pass-1).
    2. PE pass-1: z^T @ bundle[nf, 1664] -> per-row intermediates M in PSUM.
       Bundle columns are host-packed weighted contractions; the cross-product
       +/- signs are baked into the bundle so no negated copies are needed.
       Layout: [0:512)   spart: (w, q) w-major, q = feature index
               [512:1664) vpart: (k, w, j) with j in [0,3mv):
                  j<mv: C (mult v[j,k]); mv<=j<2mv: D+ (mult v[u,(k+1)%3]);
                  j>=2mv: D- (mult v[u,(k+2)%3], sign baked into bundle).
    3. Fused custom-DVE MULSCAN ops: out = running prefix sum of
       (M * broadcast z) along the stream.  One op for spart, one per k for
       vpart.  Segment sums then drop out as boundary differences of the
       prefix stream (cheap strided tensor_sub), replacing the former
       separate product ops + 1-elem/cycle tensor_reduce passes.
  Norms (si_norm/tv_norm, per-row over channels) are batched across T chunks.

  v3: the per-row quadratic forms out_w = z^T A_w z are decomposed offline
  (partially-symmetric CP / ALS, rank R1=512 / R2=896, computed from the
  runtime weights and cached):  out_w = sum_r Lam[r,w] * (U[:,r].z)^2.
  Per 4-chunk group: PE transposes z -> zT; PE MM1 (U-block stationary)
  -> PT[r,b] in PSUM; ACT squares PSUM->SBUF; PE MM2 (Lam-block) accumulates
  -> outT[w,b]; ACT copies + PE transposes back to row-major for the norms.
  DVE only runs the norm arithmetic.
"""

import sys
import numpy as np

for _p in ("/opt/trn_rl_repo",):
    if _p not in sys.path:
        sys.path.append(_p)

MI, MH, MO = 8, 16, 8
NB = 262144
NCORES = 8
ROWS_PER_CORE = NB // NCORES
P = 128
T = 32                     # chunks per macro tile
MACRO = P * T              # 4096 rows
EPS_SI = 1e-9
EPS_TV = 1e-6
TINY = 1e-12

SP = 512                   # spart region width (mw * q_s) for both layers
VK = 384                   # per-k vpart region width (mw * 3mv) for both layers
MCOLS = SP + 3 * VK        # 1664 M columns per chunk (numpy emulation only)
R1 = 512                   # CP rank, layer 1 quadratic forms (4 PE blocks)
R2 = 896                   # CP rank, layer 2 quadratic forms (7 PE blocks)
GRP = 4                    # chunks per PE/ACT group (512 batch cols)


# ---------------------------------------------------------------------------
# custom DVE op: out = prefix_sum(in0 * in1) along the free stream
# ---------------------------------------------------------------------------

_MULSCAN = None


def _get_mulscan():
    global _MULSCAN
    if _MULSCAN is not None:
        return _MULSCAN
    from concourse import dve_ops
    from concourse.dve_spec import Spec, Src0, Src1, scan, AluOp, lower, _has_src1
    from concourse.dve_uop import DveOpSpec

    name = "MULSCAN_ANT"
    for op in dve_ops.OPS:
        if op.name == name:
            _MULSCAN = op
            return op

    def _ref(in0, in1, c0, c1, c2):
        b = (np.asarray(in0, np.float32) * np.asarray(in1, np.float32))
        p = b.shape[0]
        return np.cumsum(b.reshape(p, -1), axis=-1, dtype=np.float32).reshape(b.shape)

    spec = Spec(body=scan(AluOp.ADD, Src0 * Src1), reference=_ref)
    row = dve_ops._CUSTOM_DVE_ROW_BASE + len(dve_ops.OPS)
    shas = {}
    for ver in ("v3", "v4"):
        tmp = DveOpSpec(name=name, opcode=row, uops=lower(spec, ver=ver),
                        rd1_en=_has_src1(spec))
        shas[ver] = tmp.sha(ver)
    op = dve_ops.DveOp(name, spec, subdim=False, uops_sha=shas)
    dve_ops.OPS.append(op)
    dve_ops._SUB_OPCODE_FOR_NAME[name] = row
    dve_ops.CUSTOM_DVE_SPECS[name] = spec
    _MULSCAN = op
    return op


# ---------------------------------------------------------------------------
# bundle packing
# ---------------------------------------------------------------------------

def _build_bundle(ms, mv, mw, Wss, Wvv0, Wsv, Wvs, Wvv1):
    """Pack weighted-contraction bundle [nf, MCOLS], nf = ms + 3*mv.

    z feature layout: f in [0, ms) = s_f ; f = ms + 3*u + i = v[u, i].
    spart col (w, q) = w*q_s + q  (q_s = nf):
       q in [0, ms): rows s, val c0*Wss[q, f, w]
       q = ms+3u+i:  rows v[:, i], val c0/sqrt3*Wvv0[u, f, w]
    vpart col (k, w, j) = SP + k*mw*3mv + w*3mv + j:
       j in [0, mv):  C,  rows s,            val c1/sqrt3*(Wsv[f,j,w]+Wvs[j,f,w])
       j = mv + u:    D+, rows v[:,(k+2)%3], val +c1/sqrt6*Wvv1[u,f_v,w]
       j = 2mv + u:   D-, rows v[:,(k+1)%3], val -c1/sqrt6*Wvv1[u,f_v,w]
    """
    nf = ms + 3 * mv
    q_s = nf
    c0 = (ms * ms + mv * mv) ** -0.5
    c1 = (3.0 / (2 * ms * mv + mv * mv)) ** 0.5
    inv3 = 3.0 ** -0.5
    inv6 = 6.0 ** -0.5
    assert mw * q_s == SP and mw * 3 * mv == VK
    B = np.zeros((nf, MCOLS), np.float32)
    vrows = ms + 3 * np.arange(mv)
    for w in range(mw):
        for u in range(ms):
            B[0:ms, w * q_s + u] = c0 * Wss[u, :, w]
        for u in range(mv):
            for i in range(3):
                B[vrows + i, w * q_s + ms + 3 * u + i] = c0 * inv3 * Wvv0[u, :, w]
    for k in range(3):
        i1, i2 = (k + 1) % 3, (k + 2) % 3
        for w in range(mw):
            base = SP + k * VK + w * 3 * mv
            for m in range(mv):
                B[0:ms, base + m] = c1 * inv3 * (Wsv[:, m, w] + Wvs[m, :, w])
            for u in range(mv):
                B[vrows + i2, base + mv + u] = c1 * inv6 * Wvv1[u, :, w]
                B[vrows + i1, base + 2 * mv + u] = -c1 * inv6 * Wvv1[u, :, w]
    return B


# ---------------------------------------------------------------------------
# numpy emulation of the device pipeline (for bundle/layout validation)
# ---------------------------------------------------------------------------

def _np_layer(z, bundle, ms, mv, mw):
    """Emulates MM1 + scan/diff segment sums.  yv returned K-MAJOR [n, k, w]."""
    nf = ms + 3 * mv
    n = z.shape[0]
    M = z @ bundle  # [n, MCOLS]
    v = z[:, ms:nf].reshape(n, mv, 3)
    # spart: products z_q * M[w, q], summed over q
    sM = M[:, :SP].reshape(n, mw, nf)
    out_s = np.einsum('nq,nwq->nw', z, sM)
    # vpart per k: multiplier groups [v[:,k] | v[:,(k+1)%3] | v[:,(k+2)%3]]
    out_v = np.zeros((n, 3, mw), np.float32)
    for k in range(3):
        mult = np.concatenate([v[:, :, k], v[:, :, (k + 1) % 3],
                               v[:, :, (k + 2) % 3]], axis=1)  # [n, 3mv]
        Mk = M[:, SP + k * VK: SP + (k + 1) * VK].reshape(n, mw, 3 * mv)
        out_v[:, k, :] = np.einsum('nj,nwj->nw', mult, Mk)
    return out_s, out_v  # out_v k-major


def _np_si_norm(ys, yv):
    """yv k-major [n, 3, m]."""
    n, mh = ys.shape
    std_s = ys.std(axis=1, ddof=1)
    ys = ys / (std_s[:, None] + EPS_SI)
    norm1 = np.sqrt((yv ** 2).sum(axis=1) + EPS_SI)   # [n, m]
    std_v = norm1.std(axis=1, ddof=1)
    yv = yv / (std_v[:, None, None] + EPS_SI)
    return ys, yv


def _np_tv_norm(xs, xv):
    """xv k-major [n, 3, m]."""
    xs = xs / np.sqrt((xs ** 2).sum(axis=1, keepdims=True) + EPS_TV)
    norm1 = np.sqrt((xv ** 2).sum(axis=2) + EPS_TV)   # [n, 3]
    nm = norm1.mean(axis=1)
    xv = xv / (nm[:, None, None] + EPS_TV)
    return xs, xv


def reference_numpy(x, B1, B2):
    xs = np.tanh(x[:, :MI])
    z1 = np.concatenate([xs, x[:, MI:]], axis=1)
    ys, yv = _np_layer(z1, B1, MI, MI, MH)
    ys, yv = _np_si_norm(ys, yv)
    ys, yv = _np_tv_norm(ys, yv)
    # z2 features: [s | v u-major xyz] -> from k-major yv
    z2 = np.concatenate([ys, yv.transpose(0, 2, 1).reshape(x.shape[0], -1)], axis=1)
    zs, zv = _np_layer(z2, B2, MH, MH, MO)
    zs, zv = _np_si_norm(zs, zv)
    zs = 1.0 / (1.0 + np.exp(-zs))
    return np.concatenate([zs, zv.transpose(0, 2, 1).reshape(x.shape[0], -1)],
                          axis=1).astype(np.float32)


# ---------------------------------------------------------------------------
# quadratic-form CP decomposition (partially symmetric, ALS)
# ---------------------------------------------------------------------------

def _build_quadratic_tensor(Bnd, ms, mv, mw):
    """T[p, q, w_out], w_out = [mw scalars | 3*mw vectors k-major], sym in (p,q)."""
    nf = ms + 3 * mv
    Tt = np.zeros((nf, nf, 4 * mw), np.float64)
    for w in range(mw):
        Tt[:, :, w] += Bnd[:, w * nf:(w + 1) * nf].astype(np.float64)
    for k in range(3):
        for w in range(mw):
            base = SP + k * VK + w * 3 * mv
            for j in range(3 * mv):
                if j < mv:
                    q = ms + 3 * j + k
                else:
                    u = (j - mv) % mv
                    g = 1 if j < 2 * mv else 2
                    q = ms + 3 * u + (k + g) % 3
                Tt[:, q, mw + k * mw + w] += Bnd[:, base + j].astype(np.float64)
    return 0.5 * (Tt + Tt.transpose(1, 0, 2))


def _sym_als(Tt, R, iters=250, seed=0, target=3e-4):
    """T[p,q,w] ~= sum_r u_r u_r^T lam_r."""
    rng = np.random.default_rng(seed)
    nf, _, nout = Tt.shape
    U = rng.standard_normal((nf, R)) / np.sqrt(nf)
    L = rng.standard_normal((nout, R)) * 0.1
    nT = np.linalg.norm(Tt)
    eye = np.eye(R)
    err = 1.0
    for it in range(iters):
        G = (U.T @ U) * (L.T @ L)
        rhs = np.einsum('pqw,qr,wr->pr', Tt, U, L, optimize=True)
        U = np.linalg.solve(G + 1e-10 * eye, rhs.T).T
        G2 = (U.T @ U) ** 2
        rhs2 = np.einsum('pqw,pr,qr->wr', Tt, U, U, optimize=True)
        L = np.linalg.solve(G2 + 1e-10 * eye, rhs2.T).T
        if it % 10 == 9:
            rec = np.einsum('pr,qr,wr->pqw', U, U, L, optimize=True)
            err = float(np.linalg.norm(rec - Tt) / nT)
            if err < target:
                break
    return U, L, err


_DECOMP_CACHE = {}


def _get_decomp(ws1, ws2):
    """ws1/ws2: 5-tuples of weight arrays.  Returns (U1, L1p, U2, L2p) packed
    for the device: U [nf, R] f32; Lp [128, nblocks*nout] f32 block-major."""
    key = hash(tuple(np.asarray(w).tobytes() for w in (*ws1, *ws2)))
    if key in _DECOMP_CACHE:
        return _DECOMP_CACHE[key]
    B1 = _build_bundle(MI, MI, MH, *ws1)
    B2 = _build_bundle(MH, MH, MO, *ws2)
    T1 = _build_quadratic_tensor(B1, MI, MI, MH)
    T2 = _build_quadratic_tensor(B2, MH, MH, MO)
    U1, L1, e1 = _sym_als(T1, R1, iters=250, seed=0, target=2e-4)
    U2, L2, e2 = _sym_als(T2, R2, iters=250, seed=0, target=2e-4)
    if e1 > 5e-3:  # retry with another seed if a bad local minimum was hit
        U1b, L1b, e1b = _sym_als(T1, R1, iters=250, seed=1, target=2e-4)
        if e1b < e1:
            U1, L1, e1 = U1b, L1b, e1b
    if e2 > 5e-3:
        U2b, L2b, e2b = _sym_als(T2, R2, iters=250, seed=1, target=2e-4)
        if e2b < e2:
            U2, L2, e2 = U2b, L2b, e2b

    def pack_L(L, R, nout):
        nb = R // 128
        Lp = np.zeros((128, nb * nout), np.float32)
        for rb in range(nb):
            Lp[:, rb * nout:(rb + 1) * nout] = L.T[rb * 128:(rb + 1) * 128, :]
        return Lp

    r = (np.ascontiguousarray(U1, dtype=np.float32).astype(np.float32),
         pack_L(L1, R1, 64),
         np.ascontiguousarray(U2, dtype=np.float32).astype(np.float32),
         pack_L(L2, R2, 32))
    _DECOMP_CACHE[key] = r
    return r


def _np_forward_dec(x, U1, L1p, U2, L2p):
    """Numpy forward with the decomposed layers (device-math mirror)."""
    n = x.shape[0]

    def layer(z, U, Lp, nout, R):
        nb = R // 128
        y = (z @ U) ** 2
        o = np.zeros((n, nout), np.float32)
        for rb in range(nb):
            o += y[:, rb * 128:(rb + 1) * 128] @ Lp[:, rb * nout:(rb + 1) * nout]
        mw = nout // 4
        return o[:, :mw], o[:, mw:].reshape(n, 3, mw)

    xs = np.tanh(x[:, :MI])
    z1 = np.concatenate([xs, x[:, MI:]], axis=1).astype(np.float32)
    ys, yv = layer(z1, U1, L1p, 64, R1)
    ys, yv = _np_si_norm(ys, yv)
    ys, yv = _np_tv_norm(ys, yv)
    z2 = np.concatenate([ys, yv.transpose(0, 2, 1).reshape(n, -1)],
                        axis=1).astype(np.float32)
    zs, zv = layer(z2, U2, L2p, 32, R2)
    zs, zv = _np_si_norm(zs, zv)
    zs = 1.0 / (1.0 + np.exp(-zs))
    return np.concatenate([zs, zv.transpose(0, 2, 1).reshape(n, -1)],
                          axis=1).astype(np.float32)


# ---------------------------------------------------------------------------
# device kernel
# ---------------------------------------------------------------------------

_PROGRAM_CACHE = {}


def _emit_layer_group(nc, pools, g, zsrc, nf_stride, nf, nout, nblocks,
                      U_sb, L_sb, ys_macro, yv_macro, ident, mw):
    """One group of GRP 128-row chunks of one decomposed tensor-product layer.

    zsrc:  [128, T*nf_stride] SBUF macro tile, row-major features per chunk
    U_sb:  [128, R] stationary projection (rows 0:nf used)
    L_sb:  [128, nblocks*nout] recombination blocks
    Writes ys_macro [128, T*mw] and yv_macro [128, T*3mw] (k-major) slices.
    """
    import concourse.mybir as mybir
    f32 = mybir.dt.float32
    AF = mybir.ActivationFunctionType

    # z^T for the group: 4 PE transposes -> PSUM scratch -> ACT copy to SBUF
    scr = pools["scr"].tile([P, 512], f32, tag="scr")
    for j in range(GRP):
        t = GRP * g + j
        nc.tensor.transpose(scr[0:nf, j * P:(j + 1) * P],
                            zsrc[:, t * nf_stride: t * nf_stride + nf], ident)
    ztg = pools["ztg"].tile([P, 512], f32, tag="ztg")
    nc.scalar.copy(ztg[0:nf, :], scr[0:nf, :])

    out_ps = pools["psO"].tile([P, 512], f32, tag="psO")
    sqs = []
    for rb in range(nblocks):
        pt = pools["psA"].tile([P, 512], f32, tag="psA")
        nc.tensor.matmul(pt[0:128, :], U_sb[0:nf, rb * 128:(rb + 1) * 128],
                         ztg[0:nf, :], start=True, stop=True,
                         skip_group_check=True)
        sq = pools["sq"].tile([P, 512], f32, tag="sq")
        nc.scalar.activation(sq[0:128, :], pt[0:128, :], AF.Square)
        sqs.append(sq)
        if rb >= 1:
            nc.tensor.matmul(out_ps[0:nout, :],
                             L_sb[0:128, (rb - 1) * nout:rb * nout],
                             sqs[rb - 1][0:128, :],
                             start=(rb - 1 == 0), stop=(rb - 1 == nblocks - 1),
                             skip_group_check=True)
    rb = nblocks - 1
    nc.tensor.matmul(out_ps[0:nout, :], L_sb[0:128, rb * nout:(rb + 1) * nout],
                     sqs[rb][0:128, :], start=(rb == 0), stop=True,
                     skip_group_check=True)

    # back to row-major: ACT copy out of PSUM, PE transpose per chunk
    oT = pools["oTs"].tile([P, 512], f32, tag="oTs")
    nc.scalar.copy(oT[0:nout, :], out_ps[0:nout, :])
    ymat = pools["scr"].tile([P, 512], f32, tag="scr")
    for j in range(GRP):
        nc.tensor.transpose(ymat[:, j * nout:(j + 1) * nout],
                            oT[0:nout, j * P:(j + 1) * P], ident[0:nout, 0:nout])
    ym = ymat[:, 0:GRP * nout].rearrange("p (j f) -> p j f", f=nout)
    nc.scalar.copy(
        ys_macro[:, GRP * g * mw:GRP * (g + 1) * mw].rearrange(
            "p (j w) -> p j w", w=mw),
        ym[:, :, 0:mw])
    nc.scalar.copy(
        yv_macro[:, GRP * g * 3 * mw:GRP * (g + 1) * 3 * mw].rearrange(
            "p (j f) -> p j f", f=3 * mw),
        ym[:, :, mw:4 * mw])


def _emit_program(nc, tc, x_d, u1_d, l1_d, u2_d, l2_d, out_d, rows, repeat=1):
    import concourse.mybir as mybir
    from concourse.masks import make_identity
    AF = mybir.ActivationFunctionType
    ALU = mybir.AluOpType
    AX = mybir.AxisListType
    f32 = mybir.dt.float32
    from contextlib import ExitStack

    nmacro = rows // MACRO
    ctx = ExitStack()
    with ctx:
        cpool = ctx.enter_context(tc.tile_pool(name="const", bufs=1))
        ident = cpool.tile([P, P], f32)
        make_identity(nc, ident[:])

        def _bias_tile(val, idx=[0]):
            bt = cpool.tile([P, 1], f32, tag=f"bias{idx[0]}")
            idx[0] += 1
            nc.gpsimd.memset(bt[:], float(val))
            return bt[:]

        b_tiny = _bias_tile(TINY)
        b_si = _bias_tile(EPS_SI)
        b_tv = _bias_tile(EPS_TV)
        b_v1 = _bias_tile(MH * EPS_SI / (MH - 1) + TINY)
        b_v2 = _bias_tile(MO * EPS_SI / (MO - 1) + TINY)
        u1_sb = cpool.tile([P, R1], f32)
        nc.sync.dma_start(out=u1_sb[0:32, :], in_=u1_d[:])
        l1_sb = cpool.tile([P, (R1 // 128) * 64], f32)
        nc.sync.dma_start(out=l1_sb[:], in_=l1_d[:])
        u2_sb = cpool.tile([P, R2], f32)
        nc.sync.dma_start(out=u2_sb[0:64, :], in_=u2_d[:])
        l2_sb = cpool.tile([P, (R2 // 128) * 32], f32)
        nc.sync.dma_start(out=l2_sb[:], in_=l2_d[:])

        io_pool = ctx.enter_context(tc.tile_pool(name="io", bufs=2))
        mid_pool = ctx.enter_context(tc.tile_pool(name="mid", bufs=2))
        nrm_pool = ctx.enter_context(tc.tile_pool(name="nrm", bufs=2))
        ztg_pool = ctx.enter_context(tc.tile_pool(name="ztg", bufs=2))
        sq_pool = ctx.enter_context(tc.tile_pool(name="sq", bufs=3))
        ots_pool = ctx.enter_context(tc.tile_pool(name="oTs", bufs=2))
        scr_pool = ctx.enter_context(tc.tile_pool(name="scr", bufs=2, space="PSUM"))
        psA_pool = ctx.enter_context(tc.tile_pool(name="psA", bufs=3, space="PSUM"))
        psO_pool = ctx.enter_context(tc.tile_pool(name="psO", bufs=2, space="PSUM"))
        pools = {"scr": scr_pool, "psA": psA_pool, "psO": psO_pool,
                 "ztg": ztg_pool, "sq": sq_pool, "oTs": ots_pool}

        for m in list(range(nmacro)) * repeat:
            xm = io_pool.tile([P, T * 32], f32, tag="xm")
            xview = x_d[m * MACRO:(m + 1) * MACRO, :].rearrange("(t p) f -> p t f", p=P)
            nc.sync.dma_start(out=xm[:].rearrange("p (t f) -> p t f", f=32), in_=xview)
            # tanh on scalar features (in place)
            xs_view = xm[:].rearrange("p (t f) -> p t f", f=32)[:, :, 0:MI]
            nc.scalar.activation(xs_view, xs_view, AF.Tanh)

            ys8 = nrm_pool.tile([P, T * MH], f32, tag="ys8")
            yv8 = nrm_pool.tile([P, T * 3 * MH], f32, tag="yv8")
            for g in range(T // GRP):
                _emit_layer_group(nc, pools, g, xm[:], 32, 32, 64, R1 // 128,
                                  u1_sb[:], l1_sb[:], ys8[:], yv8[:],
                                  ident[:], MH)

            # ---- si_norm(1) + tv_norm, batched over T chunks ----
            # yv8 layout per chunk: (k, u) k-major
            mh = MH
            ys8v = ys8[:].rearrange("p (t f) -> p t f", f=mh)
            yv8v = yv8[:].rearrange("p (t f) -> p t f", f=3 * mh)
            sq_s = nrm_pool.tile([P, T * mh], f32, tag="sq_s")
            nc.vector.tensor_mul(sq_s[:], ys8[:], ys8[:])
            sumsq_s = nrm_pool.tile([P, T], f32, tag="n1a")
            nc.vector.tensor_reduce(sumsq_s[:], sq_s[:].rearrange("p (t f) -> p t f", f=mh),
                                    axis=AX.X, op=ALU.add)
            sum_s = nrm_pool.tile([P, T], f32, tag="n1b")
            nc.vector.tensor_reduce(sum_s[:], ys8v, axis=AX.X, op=ALU.add)
            s2 = nrm_pool.tile([P, T], f32, tag="n1c")
            nc.scalar.activation(s2[:], sum_s[:], AF.Square, scale=float(mh) ** -0.5)
            varnum = nrm_pool.tile([P, T], f32, tag="n1d")
            nc.vector.tensor_sub(varnum[:], sumsq_s[:], s2[:])
            std_s = nrm_pool.tile([P, T], f32, tag="n1e")
            nc.scalar.activation(std_s[:], varnum[:], AF.Sqrt,
                                 scale=1.0 / (mh - 1), bias=b_tiny)
            stde_s = nrm_pool.tile([P, T], f32, tag="n1f")
            nc.vector.tensor_scalar_add(stde_s[:], std_s[:], EPS_SI)
            inv_s = nrm_pool.tile([P, T], f32, tag="n1g")
            nc.vector.reciprocal(inv_s[:], stde_s[:])

            sq_v = nrm_pool.tile([P, T * 3 * mh], f32, tag="sq_v")
            nc.vector.tensor_mul(sq_v[:], yv8[:], yv8[:])
            # n2u[t, u] = sum_k sq_v[t, k, u]  (k-major: k stride mh, u stride 1)
            n2u = nrm_pool.tile([P, T * mh], f32, tag="n2u")
            nc.vector.tensor_reduce(
                n2u[:].rearrange("p (t u) -> p t u", u=mh),
                sq_v[:].rearrange("p (t k u) -> p t u k", k=3, u=mh),
                axis=AX.X, op=ALU.add)
            norm1 = nrm_pool.tile([P, T * mh], f32, tag="norm1")
            nc.scalar.activation(norm1[:], n2u[:], AF.Sqrt, bias=b_si)
            rn = nrm_pool.tile([P, T], f32, tag="n1h")
            nc.vector.tensor_reduce(rn[:], n2u[:].rearrange("p (t u) -> p t u", u=mh),
                                    axis=AX.X, op=ALU.add)
            sum_n = nrm_pool.tile([P, T], f32, tag="n1i")
            nc.vector.tensor_reduce(sum_n[:], norm1[:].rearrange("p (t u) -> p t u", u=mh),
                                    axis=AX.X, op=ALU.add)
            s2n = nrm_pool.tile([P, T], f32, tag="n1j")
            nc.scalar.activation(s2n[:], sum_n[:], AF.Square, scale=float(mh) ** -0.5)
            varn = nrm_pool.tile([P, T], f32, tag="n1k")
            nc.vector.tensor_sub(varn[:], rn[:], s2n[:])
            std_v = nrm_pool.tile([P, T], f32, tag="n1l")
            nc.scalar.activation(std_v[:], varn[:], AF.Sqrt, scale=1.0 / (mh - 1),
                                 bias=b_v1)
            stde_v = nrm_pool.tile([P, T], f32, tag="n1m")
            nc.vector.tensor_scalar_add(stde_v[:], std_v[:], EPS_SI)
            inv_v = nrm_pool.tile([P, T], f32, tag="n1n")
            nc.vector.reciprocal(inv_v[:], stde_v[:])

            # tv_norm scalars
            invs2 = nrm_pool.tile([P, T], f32, tag="n1o")
            nc.vector.tensor_mul(invs2[:], inv_s[:], inv_s[:])
            q_sA = nrm_pool.tile([P, T], f32, tag="n1p")
            nc.vector.tensor_mul(q_sA[:], sumsq_s[:], invs2[:])
            rt_s = nrm_pool.tile([P, T], f32, tag="n1q")
            nc.scalar.activation(rt_s[:], q_sA[:], AF.Sqrt, bias=b_tv)
            invt_s = nrm_pool.tile([P, T], f32, tag="n1r")
            nc.vector.reciprocal(invt_s[:], rt_s[:])
            scale_s = nrm_pool.tile([P, T], f32, tag="n1s")
            nc.vector.tensor_mul(scale_s[:], inv_s[:], invt_s[:])

            # ni_raw[t, k] = sum_u sq_v[t, k, u]  (k-major: u innermost)
            ni_raw = nrm_pool.tile([P, T * 3], f32, tag="n1t")
            nc.vector.tensor_reduce(
                ni_raw[:].rearrange("p (t i) -> p t i", i=3),
                sq_v[:].rearrange("p (t k u) -> p t k u", k=3, u=mh),
                axis=AX.X, op=ALU.add)
            iv2 = nrm_pool.tile([P, T], f32, tag="n1u")
            nc.vector.tensor_mul(iv2[:], inv_v[:], inv_v[:])
            ni_tv = nrm_pool.tile([P, T * 3], f32, tag="n1v")
            nc.vector.tensor_mul(
                ni_tv[:].rearrange("p (t i) -> p t i", i=3),
                ni_raw[:].rearrange("p (t i) -> p t i", i=3),
                iv2[:].unsqueeze(2).broadcast_to([P, T, 3]))
            n1tv = nrm_pool.tile([P, T * 3], f32, tag="n1w")
            nc.scalar.activation(n1tv[:], ni_tv[:], AF.Sqrt, bias=b_tv)
            nm3 = nrm_pool.tile([P, T], f32, tag="n1x")
            nc.vector.tensor_reduce(nm3[:], n1tv[:].rearrange("p (t i) -> p t i", i=3),
                                    axis=AX.X, op=ALU.add)
            nme = nrm_pool.tile([P, T], f32, tag="n1y")
            nc.vector.tensor_scalar(nme[:], nm3[:], 1.0 / 3.0, EPS_TV,
                                    op0=ALU.mult, op1=ALU.add)
            invtv = nrm_pool.tile([P, T], f32, tag="n1z")
            nc.vector.reciprocal(invtv[:], nme[:])
            scale_v = nrm_pool.tile([P, T], f32, tag="n1A")
            nc.vector.tensor_mul(scale_v[:], inv_v[:], invtv[:])

            # apply scales -> zmid [128, T*64] (features [s(16) | v u-major xyz])
            zmid = mid_pool.tile([P, T * 64], f32, tag="zmid")
            zmv = zmid[:].rearrange("p (t f) -> p t f", f=64)
            nc.vector.tensor_mul(
                zmv[:, :, 0:MH], ys8v,
                scale_s[:].unsqueeze(2).broadcast_to([P, T, MH]))
            # v: out (u, k) u-major <- yv8 (k, u) k-major, times scale_v
            nc.vector.tensor_mul(
                zmv[:, :, MH:64].rearrange("p t (u k) -> p t u k", k=3, u=mh),
                yv8v.rearrange("p t (k u) -> p t u k", k=3, u=mh),
                scale_v[:].unsqueeze(2).unsqueeze(3).broadcast_to([P, T, mh, 3]))

            # ---- layer 2 ----
            zs8 = nrm_pool.tile([P, T * MO], f32, tag="zs8")
            zv8 = nrm_pool.tile([P, T * 3 * MO], f32, tag="zv8")
            for g in range(T // GRP):
                _emit_layer_group(nc, pools, g, zmid[:], 64, 64, 32, R2 // 128,
                                  u2_sb[:], l2_sb[:], zs8[:], zv8[:],
                                  ident[:], MO)

            # ---- si_norm(2) ----
            mo = MO
            zs8v = zs8[:].rearrange("p (t f) -> p t f", f=mo)
            zv8v = zv8[:].rearrange("p (t f) -> p t f", f=3 * mo)
            sq_s2 = nrm_pool.tile([P, T * mo], f32, tag="sq_s2")
            nc.vector.tensor_mul(sq_s2[:], zs8[:], zs8[:])
            sumsq2 = nrm_pool.tile([P, T], f32, tag="n2a")
            nc.vector.tensor_reduce(sumsq2[:], sq_s2[:].rearrange("p (t f) -> p t f", f=mo),
                                    axis=AX.X, op=ALU.add)
            sum2 = nrm_pool.tile([P, T], f32, tag="n2b")
            nc.vector.tensor_reduce(sum2[:], zs8v, axis=AX.X, op=ALU.add)
            s22 = nrm_pool.tile([P, T], f32, tag="n2c")
            nc.scalar.activation(s22[:], sum2[:], AF.Square, scale=float(mo) ** -0.5)
            varnum2 = nrm_pool.tile([P, T], f32, tag="n2d")
            nc.vector.tensor_sub(varnum2[:], sumsq2[:], s22[:])
            std_s2 = nrm_pool.tile([P, T], f32, tag="n2e")
            nc.scalar.activation(std_s2[:], varnum2[:], AF.Sqrt,
                                 scale=1.0 / (mo - 1), bias=b_tiny)
            stde_s2 = nrm_pool.tile([P, T], f32, tag="n2f")
            nc.vector.tensor_scalar_add(stde_s2[:], std_s2[:], EPS_SI)
            inv_s2 = nrm_pool.tile([P, T], f32, tag="n2g")
            nc.vector.reciprocal(inv_s2[:], stde_s2[:])

            sq_v2 = nrm_pool.tile([P, T * 3 * mo], f32, tag="sq_v2")
            nc.vector.tensor_mul(sq_v2[:], zv8[:], zv8[:])
            n2u2 = nrm_pool.tile([P, T * mo], f32, tag="n2u2")
            nc.vector.tensor_reduce(
                n2u2[:].rearrange("p (t u) -> p t u", u=mo),
                sq_v2[:].rearrange("p (t k u) -> p t u k", k=3, u=mo),
                axis=AX.X, op=ALU.add)
            norm12 = nrm_pool.tile([P, T * mo], f32, tag="norm12")
            nc.scalar.activation(norm12[:], n2u2[:], AF.Sqrt, bias=b_si)
            rn2 = nrm_pool.tile([P, T], f32, tag="n2h")
            nc.vector.tensor_reduce(rn2[:], n2u2[:].rearrange("p (t u) -> p t u", u=mo),
                                    axis=AX.X, op=ALU.add)
            sum_n2 = nrm_pool.tile([P, T], f32, tag="n2i")
            nc.vector.tensor_reduce(sum_n2[:], norm12[:].rearrange("p (t u) -> p t u", u=mo),
                                    axis=AX.X, op=ALU.add)
            s2n2 = nrm_pool.tile([P, T], f32, tag="n2j")
            nc.scalar.activation(s2n2[:], sum_n2[:], AF.Square, scale=float(mo) ** -0.5)
            varn2 = nrm_pool.tile([P, T], f32, tag="n2k")
            nc.vector.tensor_sub(varn2[:], rn2[:], s2n2[:])
            std_v2 = nrm_pool.tile([P, T], f32, tag="n2l")
            nc.scalar.activation(std_v2[:], varn2[:], AF.Sqrt, scale=1.0 / (mo - 1),
                                 bias=b_v2)
            stde_v2 = nrm_pool.tile([P, T], f32, tag="n2m")
            nc.vector.tensor_scalar_add(stde_v2[:], std_v2[:], EPS_SI)
            inv_v2 = nrm_pool.tile([P, T], f32, tag="n2n")
            nc.vector.reciprocal(inv_v2[:], stde_v2[:])

            # scale, sigmoid, assemble output macro [128, T*32]
            outm = io_pool.tile([P, T * 32], f32, tag="outm")
            outv = outm[:].rearrange("p (t f) -> p t f", f=32)
            tmp_s = nrm_pool.tile([P, T * mo], f32, tag="tmp_s")
            nc.vector.tensor_mul(
                tmp_s[:].rearrange("p (t f) -> p t f", f=mo), zs8v,
                inv_s2[:].unsqueeze(2).broadcast_to([P, T, mo]))
            nc.scalar.activation(outv[:, :, 0:MO],
                                 tmp_s[:].rearrange("p (t f) -> p t f", f=mo),
                                 AF.Sigmoid)
            # v out (w, k) w-major <- zv8 (k, w) k-major
            nc.vector.tensor_mul(
                outv[:, :, MO:32].rearrange("p t (w k) -> p t w k", k=3, w=mo),
                zv8v.rearrange("p t (k w) -> p t w k", k=3, w=mo),
                inv_v2[:].unsqueeze(2).unsqueeze(3).broadcast_to([P, T, mo, 3]))

            oview = out_d[m * MACRO:(m + 1) * MACRO, :].rearrange("(t p) f -> p t f", p=P)
            nc.sync.dma_start(out=oview, in_=outm[:].rearrange("p (t f) -> p t f", f=32))


def _build_program(rows, repeat=1):
    import concourse.bacc as bacc
    import concourse.tile as tile
    import concourse.mybir as mybir
    f32 = mybir.dt.float32

    nc = bacc.Bacc("TRN2", target_bir_lowering=False, debug=False,
                   enable_asserts=False, num_devices=NCORES)
    x_d = nc.dram_tensor("x", [rows, 32], f32, kind="ExternalInput").ap()
    u1_d = nc.dram_tensor("u1", [32, R1], f32, kind="ExternalInput").ap()
    l1_d = nc.dram_tensor("l1", [128, (R1 // 128) * 64], f32, kind="ExternalInput").ap()
    u2_d = nc.dram_tensor("u2", [64, R2], f32, kind="ExternalInput").ap()
    l2_d = nc.dram_tensor("l2", [128, (R2 // 128) * 32], f32, kind="ExternalInput").ap()
    out_d = nc.dram_tensor("out", [rows, 32], f32, kind="ExternalOutput").ap()

    with tile.TileContext(nc) as tc:
        _emit_program(nc, tc, x_d, u1_d, l1_d, u2_d, l2_d, out_d, rows, repeat)
    nc.compile()
    return nc


def _get_program(rows, repeat=1):
    key = (rows, repeat)
    if key not in _PROGRAM_CACHE:
        _PROGRAM_CACHE[key] = _build_program(rows, repeat)
    return _PROGRAM_CACHE[key]


_RUN_CACHE = {}


def _get_runner(rows, repeat):
    """Build (once) a cached jitted shard_map executable for the program."""
    key = (rows, repeat)
    if key in _RUN_CACHE:
        return _RUN_CACHE[key]
    import jax
    import numpy as _np
    from jax.sharding import Mesh, PartitionSpec
    try:
        from jax.experimental.shard_map import shard_map
    except Exception:
        from jax.shard_map import shard_map  # newer jax
    from concourse import bass2jax
    import concourse.mybir as mybir

    nc = _get_program(rows, repeat)
    bass2jax.install_neuronx_cc_hook()
    partition_name = nc.partition_id_tensor.name if nc.partition_id_tensor else None
    in_names, out_names, out_avals, zero_outs = [], [], [], []
    for alloc in nc.m.functions[0].allocations:
        if not isinstance(alloc, mybir.MemoryLocationSet):
            continue
        name = alloc.memorylocations[0].name
        if alloc.kind == "ExternalInput":
            if name != partition_name:
                in_names.append(name)
        elif alloc.kind == "ExternalOutput":
            shape = tuple(alloc.tensor_shape)
            dtype = mybir.dt.np(alloc.dtype)
            out_names.append(name)
            out_avals.append(jax.core.ShapedArray(shape, dtype))
            zero_outs.append(_np.zeros(shape, dtype))
    n_params = len(in_names)
    n_outs = len(out_avals)
    all_in_names = list(in_names) + list(out_names)
    if partition_name is not None:
        all_in_names.append(partition_name)
    donate = tuple(range(n_params, n_params + n_outs))

    def _body(*args):
        operands = list(args)
        if partition_name is not None:
            operands.append(bass2jax.partition_id_tensor())
        outs = bass2jax._bass_exec_p.bind(
            *operands,
            out_avals=tuple(out_avals),
            in_names=tuple(all_in_names),
            out_names=tuple(out_names),
            lowering_input_output_aliases=(),
            sim_require_finite=True,
            sim_require_nnan=True,
            nc=nc,
        )
        return tuple(outs)

    devices = jax.devices()[:NCORES]
    mesh = Mesh(_np.asarray(devices), ("core",))
    in_specs = (PartitionSpec("core"),) * (n_params + n_outs)
    out_specs = (PartitionSpec("core"),) * n_outs
    sharded = jax.jit(
        shard_map(_body, mesh=mesh, in_specs=in_specs, out_specs=out_specs,
                  check_rep=False),
        donate_argnums=donate, keep_unused=True,
    )
    runner = (sharded, in_names, out_names, out_avals, zero_outs)
    _RUN_CACHE[key] = runner
    return runner


_ZERO_CACHE = {}


def _run_cached(rows, repeat, full_inputs):
    """full_inputs: dict name -> already-concatenated (NCORES*rows0, ...) array."""
    import numpy as _np
    sharded, in_names, out_names, out_avals, zero_outs = _get_runner(rows, repeat)
    concat_in = [full_inputs[nm] for nm in in_names]
    key = (rows, repeat)
    if key not in _ZERO_CACHE:
        _ZERO_CACHE[key] = [
            _np.zeros((NCORES * z.shape[0], *z.shape[1:]), z.dtype)
            for z in zero_outs]
    out_arrs = sharded(*concat_in, *_ZERO_CACHE[key])
    i = out_names.index("out")
    return _np.asarray(out_arrs[i]).reshape(NCORES, *out_avals[i].shape)


def kernel(x, w1_ss, w1_vv0, w1_sv, w1_vs, w1_vv1,
           w2_ss, w2_vv0, w2_sv, w2_vs, w2_vv1, _trace=False, _repeat=1):
    from concourse import bass_utils

    x = np.asarray(x, dtype=np.float32)
    U1, L1p, U2, L2p = _get_decomp(
        (np.asarray(w1_ss), np.asarray(w1_vv0), np.asarray(w1_sv),
         np.asarray(w1_vs), np.asarray(w1_vv1)),
        (np.asarray(w2_ss), np.asarray(w2_vv0), np.asarray(w2_sv),
         np.asarray(w2_vs), np.asarray(w2_vv1)))

    rows = x.shape[0] // NCORES
    if _trace:
        shards = x.reshape(NCORES, rows, 32)
        in_maps = [{"x": np.ascontiguousarray(shards[i]), "u1": U1, "l1": L1p,
                    "u2": U2, "l2": L2p}
                   for i in range(NCORES)]
        nc = _get_program(rows, _repeat)
        res = bass_utils.run_bass_kernel_spmd(nc, in_maps,
                                              core_ids=list(range(NCORES)),
                                              trace=True)
        out = np.concatenate([res.results[i]["out"] for i in range(NCORES)], axis=0)
        return out, res
    full_inputs = {"x": np.ascontiguousarray(x),
                   "u1": np.tile(U1, (NCORES, 1)),
                   "l1": np.tile(L1p, (NCORES, 1)),
                   "u2": np.tile(U2, (NCORES, 1)),
                   "l2": np.tile(L2p, (NCORES, 1))}
    try:
        per_core = _run_cached(rows, _repeat, full_inputs)
        return np.ascontiguousarray(per_core.reshape(rows * NCORES, 32))
    except Exception:
        shards = x.reshape(NCORES, rows, 32)
        in_maps = [{"x": np.ascontiguousarray(shards[i]), "u1": U1, "l1": L1p,
                    "u2": U2, "l2": L2p}
                   for i in range(NCORES)]
        nc = _get_program(rows, _repeat)
        res = bass_utils.run_bass_kernel_spmd(nc, in_maps,
                                              core_ids=list(range(NCORES)))
        return np.concatenate([res.results[i]["out"] for i in range(NCORES)], axis=0)


# revision 11
# speedup vs baseline: 4.5338x; 3.2949x over previous
"""Trainium2 Bass kernel for nn_DoubleLayer (e3nn-style double tensor-product layer).

Math per row b (layout x = [s(8) | v(8 vec channels, u-major xyz)]):
  layer(s, v; W) with irreps (ms x 0e + mv x 1o) -> (mw x 0e + mw x 1o):
    out_s[w]   = c0*(sum_uv s_u s_v Wss[u,v,w] + 1/sqrt3 * sum_uvi v_ui v_vi Wvv0[u,v,w])
    out_v[w,k] = c1*(1/sqrt3*(sum_uv s_u v_vk Wsv[u,v,w] + v_uk s_v Wvs[u,v,w])
                 + 1/sqrt6 * sum eps_ijk v_ui v_vj Wvv1[u,v,w])
  x -> tanh(s),v -> L1 -> si_norm -> tv_norm -> L2 -> si_norm -> sigmoid(s).

Kernel strategy v2 (pure data parallel over 8 cores, 32768 rows/core):
  For each 128-row chunk (batch rows on SBUF partitions):
    1. PE transpose z [128, nf] -> z^T (into a scratch region of the PSUM
       matmul tile); ACT copies it to SBUF (stationary for pass-1).
    2. PE pass-1: z^T @ bundle[nf, 1664] -> per-row intermediates M in PSUM.
       Bundle columns are host-packed weighted contractions; the cross-product
       +/- signs are baked into the bundle so no negated copies are needed.
       Layout: [0:512)   spart: (w, q) w-major, q = feature index
               [512:1664) vpart: (k, w, j) with j in [0,3mv):
                  j<mv: C (mult v[j,k]); mv<=j<2mv: D+ (mult v[u,(k+1)%3]);
                  j>=2mv: D- (mult v[u,(k+2)%3], sign baked into bundle).
    3. Fused custom-DVE MULSCAN ops: out = running prefix sum of
       (M * broadcast z) along the stream.  One op for spart, one per k for
       vpart.  Segment sums then drop out as boundary differences of the
       prefix stream (cheap strided tensor_sub), replacing the former
       separate product ops + 1-elem/cycle tensor_reduce passes.
  Norms (si_norm/tv_norm, per-row over channels) are batched across T chunks.

  v3: the per-row quadratic forms out_w = z^T A_w z are decomposed offline
  (partially-symmetric CP / ALS, rank R1=512 / R2=896, computed from the
  runtime weights and cached):  out_w = sum_r Lam[r,w] * (U[:,r].z)^2.
  Per 4-chunk group: PE transposes z -> zT; PE MM1 (U-block stationary)
  -> PT[r,b] in PSUM; ACT squares PSUM->SBUF; PE MM2 (Lam-block) accumulates
  -> outT[w,b]; ACT copies + PE transposes back to row-major for the norms.
  DVE only runs the norm arithmetic.
"""

import sys
import numpy as np

for _p in ("/opt/trn_rl_repo",):
    if _p not in sys.path:
        sys.path.append(_p)

MI, MH, MO = 8, 16, 8
NB = 262144
NCORES = 8
ROWS_PER_CORE = NB // NCORES
P = 128
T = 32                     # chunks per macro tile
MACRO = P * T              # 4096 rows
EPS_SI = 1e-9
EPS_TV = 1e-6
TINY = 1e-12

SP = 512                   # spart region width (mw * q_s) for both layers
VK = 384                   # per-k vpart region width (mw * 3mv) for both layers
MCOLS = SP + 3 * VK        # 1664 M columns per chunk (numpy emulation only)
R1 = 512                   # CP rank, layer 1 quadratic forms (4 PE blocks)
R2 = 896                   # CP rank, layer 2 quadratic forms (7 PE blocks)
GRP = 4                    # chunks per PE/ACT group (512 batch cols)


# ---------------------------------------------------------------------------
# custom DVE op: out = prefix_sum(in0 * in1) along the free stream
# ---------------------------------------------------------------------------

_MULSCAN = None


def _get_mulscan():
    global _MULSCAN
    if _MULSCAN is not None:
        return _MULSCAN
    from concourse import dve_ops
    from concourse.dve_spec import Spec, Src0, Src1, scan, AluOp, lower, _has_src1
    from concourse.dve_uop import DveOpSpec

    name = "MULSCAN_ANT"
    for op in dve_ops.OPS:
        if op.name == name:
            _MULSCAN = op
            return op

    def _ref(in0, in1, c0, c1, c2):
        b = (np.asarray(in0, np.float32) * np.asarray(in1, np.float32))
        p = b.shape[0]
        return np.cumsum(b.reshape(p, -1), axis=-1, dtype=np.float32).reshape(b.shape)

    spec = Spec(body=scan(AluOp.ADD, Src0 * Src1), reference=_ref)
    row = dve_ops._CUSTOM_DVE_ROW_BASE + len(dve_ops.OPS)
    shas = {}
    for ver in ("v3", "v4"):
        tmp = DveOpSpec(name=name, opcode=row, uops=lower(spec, ver=ver),
                        rd1_en=_has_src1(spec))
        shas[ver] = tmp.sha(ver)
    op = dve_ops.DveOp(name, spec, subdim=False, uops_sha=shas)
    dve_ops.OPS.append(op)
    dve_ops._SUB_OPCODE_FOR_NAME[name] = row
    dve_ops.CUSTOM_DVE_SPECS[name] = spec
    _MULSCAN = op
    return op


# ---------------------------------------------------------------------------
# bundle packing
# ---------------------------------------------------------------------------

def _build_bundle(ms, mv, mw, Wss, Wvv0, Wsv, Wvs, Wvv1):
    """Pack weighted-contraction bundle [nf, MCOLS], nf = ms + 3*mv.

    z feature layout: f in [0, ms) = s_f ; f = ms + 3*u + i = v[u, i].
    spart col (w, q) = w*q_s + q  (q_s = nf):
       q in [0, ms): rows s, val c0*Wss[q, f, w]
       q = ms+3u+i:  rows v[:, i], val c0/sqrt3*Wvv0[u, f, w]
    vpart col (k, w, j) = SP + k*mw*3mv + w*3mv + j:
       j in [0, mv):  C,  rows s,            val c1/sqrt3*(Wsv[f,j,w]+Wvs[j,f,w])
       j = mv + u:    D+, rows v[:,(k+2)%3], val +c1/sqrt6*Wvv1[u,f_v,w]
       j = 2mv + u:   D-, rows v[:,(k+1)%3], val -c1/sqrt6*Wvv1[u,f_v,w]
    """
    nf = ms + 3 * mv
    q_s = nf
    c0 = (ms * ms + mv * mv) ** -0.5
    c1 = (3.0 / (2 * ms * mv + mv * mv)) ** 0.5
    inv3 = 3.0 ** -0.5
    inv6 = 6.0 ** -0.5
    assert mw * q_s == SP and mw * 3 * mv == VK
    B = np.zeros((nf, MCOLS), np.float32)
    vrows = ms + 3 * np.arange(mv)
    for w in range(mw):
        for u in range(ms):
            B[0:ms, w * q_s + u] = c0 * Wss[u, :, w]
        for u in range(mv):
            for i in range(3):
                B[vrows + i, w * q_s + ms + 3 * u + i] = c0 * inv3 * Wvv0[u, :, w]
    for k in range(3):
        i1, i2 = (k + 1) % 3, (k + 2) % 3
        for w in range(mw):
            base = SP + k * VK + w * 3 * mv
            for m in range(mv):
                B[0:ms, base + m] = c1 * inv3 * (Wsv[:, m, w] + Wvs[m, :, w])
            for u in range(mv):
                B[vrows + i2, base + mv + u] = c1 * inv6 * Wvv1[u, :, w]
                B[vrows + i1, base + 2 * mv + u] = -c1 * inv6 * Wvv1[u, :, w]
    return B


# ---------------------------------------------------------------------------
# numpy emulation of the device pipeline (for bundle/layout validation)
# ---------------------------------------------------------------------------

def _np_layer(z, bundle, ms, mv, mw):
    """Emulates MM1 + scan/diff segment sums.  yv returned K-MAJOR [n, k, w]."""
    nf = ms + 3 * mv
    n = z.shape[0]
    M = z @ bundle  # [n, MCOLS]
    v = z[:, ms:nf].reshape(n, mv, 3)
    # spart: products z_q * M[w, q], summed over q
    sM = M[:, :SP].reshape(n, mw, nf)
    out_s = np.einsum('nq,nwq->nw', z, sM)
    # vpart per k: multiplier groups [v[:,k] | v[:,(k+1)%3] | v[:,(k+2)%3]]
    out_v = np.zeros((n, 3, mw), np.float32)
    for k in range(3):
        mult = np.concatenate([v[:, :, k], v[:, :, (k + 1) % 3],
                               v[:, :, (k + 2) % 3]], axis=1)  # [n, 3mv]
        Mk = M[:, SP + k * VK: SP + (k + 1) * VK].reshape(n, mw, 3 * mv)
        out_v[:, k, :] = np.einsum('nj,nwj->nw', mult, Mk)
    return out_s, out_v  # out_v k-major


def _np_si_norm(ys, yv):
    """yv k-major [n, 3, m]."""
    n, mh = ys.shape
    std_s = ys.std(axis=1, ddof=1)
    ys = ys / (std_s[:, None] + EPS_SI)
    norm1 = np.sqrt((yv ** 2).sum(axis=1) + EPS_SI)   # [n, m]
    std_v = norm1.std(axis=1, ddof=1)
    yv = yv / (std_v[:, None, None] + EPS_SI)
    return ys, yv


def _np_tv_norm(xs, xv):
    """xv k-major [n, 3, m]."""
    xs = xs / np.sqrt((xs ** 2).sum(axis=1, keepdims=True) + EPS_TV)
    norm1 = np.sqrt((xv ** 2).sum(axis=2) + EPS_TV)   # [n, 3]
    nm = norm1.mean(axis=1)
    xv = xv / (nm[:, None, None] + EPS_TV)
    return xs, xv


def reference_numpy(x, B1, B2):
    xs = np.tanh(x[:, :MI])
    z1 = np.concatenate([xs, x[:, MI:]], axis=1)
    ys, yv = _np_layer(z1, B1, MI, MI, MH)
    ys, yv = _np_si_norm(ys, yv)
    ys, yv = _np_tv_norm(ys, yv)
    # z2 features: [s | v u-major xyz] -> from k-major yv
    z2 = np.concatenate([ys, yv.transpose(0, 2, 1).reshape(x.shape[0], -1)], axis=1)
    zs, zv = _np_layer(z2, B2, MH, MH, MO)
    zs, zv = _np_si_norm(zs, zv)
    zs = 1.0 / (1.0 + np.exp(-zs))
    return np.concatenate([zs, zv.transpose(0, 2, 1).reshape(x.shape[0], -1)],
                          axis=1).astype(np.float32)


# ---------------------------------------------------------------------------
# quadratic-form CP decomposition (partially symmetric, ALS)
# ---------------------------------------------------------------------------

def _build_quadratic_tensor(Bnd, ms, mv, mw):
    """T[p, q, w_out], w_out = [mw scalars | 3*mw vectors k-major], sym in (p,q)."""
    nf = ms + 3 * mv
    Tt = np.zeros((nf, nf, 4 * mw), np.float64)
    for w in range(mw):
        Tt[:, :, w] += Bnd[:, w * nf:(w + 1) * nf].astype(np.float64)
    for k in range(3):
        for w in range(mw):
            base = SP + k * VK + w * 3 * mv
            for j in range(3 * mv):
                if j < mv:
                    q = ms + 3 * j + k
                else:
                    u = (j - mv) % mv
                    g = 1 if j < 2 * mv else 2
                    q = ms + 3 * u + (k + g) % 3
                Tt[:, q, mw + k * mw + w] += Bnd[:, base + j].astype(np.float64)
    return 0.5 * (Tt + Tt.transpose(1, 0, 2))


def _sym_als(Tt, R, iters=250, seed=0, target=3e-4):
    """T[p,q,w] ~= sum_r u_r u_r^T lam_r."""
    rng = np.random.default_rng(seed)
    nf, _, nout = Tt.shape
    U = rng.standard_normal((nf, R)) / np.sqrt(nf)
    L = rng.standard_normal((nout, R)) * 0.1
    nT = np.linalg.norm(Tt)
    eye = np.eye(R)
    err = 1.0
    for it in range(iters):
        G = (U.T @ U) * (L.T @ L)
        rhs = np.einsum('pqw,qr,wr->pr', Tt, U, L, optimize=True)
        U = np.linalg.solve(G + 1e-10 * eye, rhs.T).T
        G2 = (U.T @ U) ** 2
        rhs2 = np.einsum('pqw,pr,qr->wr', Tt, U, U, optimize=True)
        L = np.linalg.solve(G2 + 1e-10 * eye, rhs2.T).T
        if it % 10 == 9:
            rec = np.einsum('pr,qr,wr->pqw', U, U, L, optimize=True)
            err = float(np.linalg.norm(rec - Tt) / nT)
            if err < target:
                break
    return U, L, err


_DECOMP_CACHE = {}


def _get_decomp(ws1, ws2):
    """ws1/ws2: 5-tuples of weight arrays.  Returns (U1, L1p, U2, L2p) packed
    for the device: U [nf, R] f32; Lp [128, nblocks*nout] f32 block-major."""
    key = hash(tuple(np.asarray(w).tobytes() for w in (*ws1, *ws2)))
    if key in _DECOMP_CACHE:
        return _DECOMP_CACHE[key]
    B1 = _build_bundle(MI, MI, MH, *ws1)
    B2 = _build_bundle(MH, MH, MO, *ws2)
    T1 = _build_quadratic_tensor(B1, MI, MI, MH)
    T2 = _build_quadratic_tensor(B2, MH, MH, MO)
    U1, L1, e1 = _sym_als(T1, R1, iters=250, seed=0, target=2e-4)
    U2, L2, e2 = _sym_als(T2, R2, iters=250, seed=0, target=2e-4)
    if e1 > 5e-3:  # retry with another seed if a bad local minimum was hit
        U1b, L1b, e1b = _sym_als(T1, R1, iters=250, seed=1, target=2e-4)
        if e1b < e1:
            U1, L1, e1 = U1b, L1b, e1b
    if e2 > 5e-3:
        U2b, L2b, e2b = _sym_als(T2, R2, iters=250, seed=1, target=2e-4)
        if e2b < e2:
            U2, L2, e2 = U2b, L2b, e2b

    def pack_L(L, R, nout):
        nb = R // 128
        Lp = np.zeros((128, nb * nout), np.float32)
        for rb in range(nb):
            Lp[:, rb * nout:(rb + 1) * nout] = L.T[rb * 128:(rb + 1) * 128, :]
        return Lp

    r = (np.ascontiguousarray(U1, dtype=np.float32).astype(np.float32),
         pack_L(L1, R1, 64),
         np.ascontiguousarray(U2, dtype=np.float32).astype(np.float32),
         pack_L(L2, R2, 32))
    _DECOMP_CACHE[key] = r
    return r


def _np_forward_dec(x, U1, L1p, U2, L2p):
    """Numpy forward with the decomposed layers (device-math mirror)."""
    n = x.shape[0]

    def layer(z, U, Lp, nout, R):
        nb = R // 128
        y = (z @ U) ** 2
        o = np.zeros((n, nout), np.float32)
        for rb in range(nb):
            o += y[:, rb * 128:(rb + 1) * 128] @ Lp[:, rb * nout:(rb + 1) * nout]
        mw = nout // 4
        return o[:, :mw], o[:, mw:].reshape(n, 3, mw)

    xs = np.tanh(x[:, :MI])
    z1 = np.concatenate([xs, x[:, MI:]], axis=1).astype(np.float32)
    ys, yv = layer(z1, U1, L1p, 64, R1)
    ys, yv = _np_si_norm(ys, yv)
    ys, yv = _np_tv_norm(ys, yv)
    z2 = np.concatenate([ys, yv.transpose(0, 2, 1).reshape(n, -1)],
                        axis=1).astype(np.float32)
    zs, zv = layer(z2, U2, L2p, 32, R2)
    zs, zv = _np_si_norm(zs, zv)
    zs = 1.0 / (1.0 + np.exp(-zs))
    return np.concatenate([zs, zv.transpose(0, 2, 1).reshape(n, -1)],
                          axis=1).astype(np.float32)


# ---------------------------------------------------------------------------
# device kernel
# ---------------------------------------------------------------------------

_PROGRAM_CACHE = {}


def _emit_layer_group(nc, pools, g, zsrc, nf_stride, nf, nout, nblocks,
                      U_sb, L_sb, ys_macro, yv_macro, ident, mw):
    """One group of GRP 128-row chunks of one decomposed tensor-product layer.

    zsrc:  [128, T*nf_stride] SBUF macro tile, row-major features per chunk
    U_sb:  [128, R] stationary projection (rows 0:nf used)
    L_sb:  [128, nblocks*nout] recombination blocks
    Writes ys_macro [128, T*mw] and yv_macro [128, T*3mw] (k-major) slices.
    """
    import concourse.mybir as mybir
    f32 = mybir.dt.float32
    AF = mybir.ActivationFunctionType

    # z^T for the group: 4 PE transposes -> PSUM scratch -> ACT copy to SBUF
    scr = pools["scr"].tile([P, 512], f32, tag="scr")
    for j in range(GRP):
        t = GRP * g + j
        nc.tensor.transpose(scr[0:nf, j * P:(j + 1) * P],
                            zsrc[:, t * nf_stride: t * nf_stride + nf], ident)
    ztg = pools["ztg"].tile([P, 512], f32, tag="ztg")
    nc.scalar.copy(ztg[0:nf, :], scr[0:nf, :])

    out_ps = pools["psO"].tile([P, 512], f32, tag="psO")
    sqs = []
    for rb in range(nblocks):
        pt = pools["psA"].tile([P, 512], f32, tag="psA")
        nc.tensor.matmul(pt[0:128, :], U_sb[0:nf, rb * 128:(rb + 1) * 128],
                         ztg[0:nf, :], start=True, stop=True,
                         skip_group_check=True)
        sq = pools["sq"].tile([P, 512], f32, tag="sq")
        nc.scalar.activation(sq[0:128, :], pt[0:128, :], AF.Square)
        sqs.append(sq)
        if rb >= 1:
            nc.tensor.matmul(out_ps[0:nout, :],
                             L_sb[0:128, (rb - 1) * nout:rb * nout],
                             sqs[rb - 1][0:128, :],
                             start=(rb - 1 == 0), stop=(rb - 1 == nblocks - 1),
                             skip_group_check=True)
    rb = nblocks - 1
    nc.tensor.matmul(out_ps[0:nout, :], L_sb[0:128, rb * nout:(rb + 1) * nout],
                     sqs[rb][0:128, :], start=(rb == 0), stop=True,
                     skip_group_check=True)

    # back to row-major: ACT copy out of PSUM, PE transpose per chunk
    oT = pools["oTs"].tile([P, 512], f32, tag="oTs")
    nc.vector.tensor_copy(oT[0:nout, :], out_ps[0:nout, :])
    ymat = pools["scr"].tile([P, 512], f32, tag="scr")
    for j in range(GRP):
        nc.tensor.transpose(ymat[:, j * nout:(j + 1) * nout],
                            oT[0:nout, j * P:(j + 1) * P], ident[0:nout, 0:nout])
    ym = ymat[:, 0:GRP * nout].rearrange("p (j f) -> p j f", f=nout)
    nc.vector.tensor_copy(
        ys_macro[:, GRP * g * mw:GRP * (g + 1) * mw].rearrange(
            "p (j w) -> p j w", w=mw),
        ym[:, :, 0:mw])
    nc.vector.tensor_copy(
        yv_macro[:, GRP * g * 3 * mw:GRP * (g + 1) * 3 * mw].rearrange(
            "p (j f) -> p j f", f=3 * mw),
        ym[:, :, mw:4 * mw])


def _emit_program(nc, tc, x_d, u1_d, l1_d, u2_d, l2_d, out_d, rows, repeat=1):
    import concourse.mybir as mybir
    from concourse.masks import make_identity
    AF = mybir.ActivationFunctionType
    ALU = mybir.AluOpType
    AX = mybir.AxisListType
    f32 = mybir.dt.float32
    from contextlib import ExitStack

    nmacro = rows // MACRO
    ctx = ExitStack()
    with ctx:
        cpool = ctx.enter_context(tc.tile_pool(name="const", bufs=1))
        ident = cpool.tile([P, P], f32)
        make_identity(nc, ident[:])

        def _bias_tile(val, idx=[0]):
            bt = cpool.tile([P, 1], f32, tag=f"bias{idx[0]}")
            idx[0] += 1
            nc.gpsimd.memset(bt[:], float(val))
            return bt[:]

        b_tiny = _bias_tile(TINY)
        b_si = _bias_tile(EPS_SI)
        b_tv = _bias_tile(EPS_TV)
        b_v1 = _bias_tile(MH * EPS_SI / (MH - 1) + TINY)
        b_v2 = _bias_tile(MO * EPS_SI / (MO - 1) + TINY)
        u1_sb = cpool.tile([P, R1], f32)
        nc.sync.dma_start(out=u1_sb[0:32, :], in_=u1_d[:])
        l1_sb = cpool.tile([P, (R1 // 128) * 64], f32)
        nc.sync.dma_start(out=l1_sb[:], in_=l1_d[:])
        u2_sb = cpool.tile([P, R2], f32)
        nc.sync.dma_start(out=u2_sb[0:64, :], in_=u2_d[:])
        l2_sb = cpool.tile([P, (R2 // 128) * 32], f32)
        nc.sync.dma_start(out=l2_sb[:], in_=l2_d[:])

        io_pool = ctx.enter_context(tc.tile_pool(name="io", bufs=2))
        mid_pool = ctx.enter_context(tc.tile_pool(name="mid", bufs=2))
        nrm_pool = ctx.enter_context(tc.tile_pool(name="nrm", bufs=2))
        ztg_pool = ctx.enter_context(tc.tile_pool(name="ztg", bufs=3))
        sq_pool = ctx.enter_context(tc.tile_pool(name="sq", bufs=5))
        ots_pool = ctx.enter_context(tc.tile_pool(name="oTs", bufs=3))
        scr_pool = ctx.enter_context(tc.tile_pool(name="scr", bufs=3, space="PSUM"))
        psA_pool = ctx.enter_context(tc.tile_pool(name="psA", bufs=3, space="PSUM"))
        psO_pool = ctx.enter_context(tc.tile_pool(name="psO", bufs=2, space="PSUM"))
        pools = {"scr": scr_pool, "psA": psA_pool, "psO": psO_pool,
                 "ztg": ztg_pool, "sq": sq_pool, "oTs": ots_pool}

        for m in list(range(nmacro)) * repeat:
            xm = io_pool.tile([P, T * 32], f32, tag="xm")
            xview = x_d[m * MACRO:(m + 1) * MACRO, :].rearrange("(t p) f -> p t f", p=P)
            nc.sync.dma_start(out=xm[:].rearrange("p (t f) -> p t f", f=32), in_=xview)
            # tanh on scalar features (in place)
            xs_view = xm[:].rearrange("p (t f) -> p t f", f=32)[:, :, 0:MI]
            nc.scalar.activation(xs_view, xs_view, AF.Tanh)

            ys8 = nrm_pool.tile([P, T * MH], f32, tag="ys8")
            yv8 = nrm_pool.tile([P, T * 3 * MH], f32, tag="yv8")
            for g in range(T // GRP):
                _emit_layer_group(nc, pools, g, xm[:], 32, 32, 64, R1 // 128,
                                  u1_sb[:], l1_sb[:], ys8[:], yv8[:],
                                  ident[:], MH)

            # ---- si_norm(1) + tv_norm, batched over T chunks ----
            # yv8 layout per chunk: (k, u) k-major
            mh = MH
            ys8v = ys8[:].rearrange("p (t f) -> p t f", f=mh)
            yv8v = yv8[:].rearrange("p (t f) -> p t f", f=3 * mh)
            sq_s = nrm_pool.tile([P, T * mh], f32, tag="sq_s")
            nc.vector.tensor_mul(sq_s[:], ys8[:], ys8[:])
            sumsq_s = nrm_pool.tile([P, T], f32, tag="n1a")
            nc.vector.tensor_reduce(sumsq_s[:], sq_s[:].rearrange("p (t f) -> p t f", f=mh),
                                    axis=AX.X, op=ALU.add)
            sum_s = nrm_pool.tile([P, T], f32, tag="n1b")
            nc.vector.tensor_reduce(sum_s[:], ys8v, axis=AX.X, op=ALU.add)
            s2 = nrm_pool.tile([P, T], f32, tag="n1c")
            nc.scalar.activation(s2[:], sum_s[:], AF.Square, scale=float(mh) ** -0.5)
            varnum = nrm_pool.tile([P, T], f32, tag="n1d")
            nc.vector.tensor_sub(varnum[:], sumsq_s[:], s2[:])
            std_s = nrm_pool.tile([P, T], f32, tag="n1e")
            nc.scalar.activation(std_s[:], varnum[:], AF.Sqrt,
                                 scale=1.0 / (mh - 1), bias=b_tiny)
            stde_s = nrm_pool.tile([P, T], f32, tag="n1f")
            nc.vector.tensor_scalar_add(stde_s[:], std_s[:], EPS_SI)
            inv_s = nrm_pool.tile([P, T], f32, tag="n1g")
            nc.vector.reciprocal(inv_s[:], stde_s[:])

            sq_v = nrm_pool.tile([P, T * 3 * mh], f32, tag="sq_v")
            nc.vector.tensor_mul(sq_v[:], yv8[:], yv8[:])
            # n2u[t, u] = sum_k sq_v[t, k, u]  (k-major: k stride mh, u stride 1)
            n2u = nrm_pool.tile([P, T * mh], f32, tag="n2u")
            nc.vector.tensor_reduce(
                n2u[:].rearrange("p (t u) -> p t u", u=mh),
                sq_v[:].rearrange("p (t k u) -> p t u k", k=3, u=mh),
                axis=AX.X, op=ALU.add)
            norm1 = nrm_pool.tile([P, T * mh], f32, tag="norm1")
            nc.scalar.activation(norm1[:], n2u[:], AF.Sqrt, bias=b_si)
            rn = nrm_pool.tile([P, T], f32, tag="n1h")
            nc.vector.tensor_reduce(rn[:], n2u[:].rearrange("p (t u) -> p t u", u=mh),
                                    axis=AX.X, op=ALU.add)
            sum_n = nrm_pool.tile([P, T], f32, tag="n1i")
            nc.vector.tensor_reduce(sum_n[:], norm1[:].rearrange("p (t u) -> p t u", u=mh),
                                    axis=AX.X, op=ALU.add)
            s2n = nrm_pool.tile([P, T], f32, tag="n1j")
            nc.scalar.activation(s2n[:], sum_n[:], AF.Square, scale=float(mh) ** -0.5)
            varn = nrm_pool.tile([P, T], f32, tag="n1k")
            nc.vector.tensor_sub(varn[:], rn[:], s2n[:])
            std_v = nrm_pool.tile([P, T], f32, tag="n1l")
            nc.scalar.activation(std_v[:], varn[:], AF.Sqrt, scale=1.0 / (mh - 1),
                                 bias=b_v1)
            stde_v = nrm_pool.tile([P, T], f32, tag="n1m")
            nc.vector.tensor_scalar_add(stde_v[:], std_v[:], EPS_SI)
            inv_v = nrm_pool.tile([P, T], f32, tag="n1n")
            nc.vector.reciprocal(inv_v[:], stde_v[:])

            # tv_norm scalars
            invs2 = nrm_pool.tile([P, T], f32, tag="n1o")
            nc.vector.tensor_mul(invs2[:], inv_s[:], inv_s[:])
            q_sA = nrm_pool.tile([P, T], f32, tag="n1p")
            nc.vector.tensor_mul(q_sA[:], sumsq_s[:], invs2[:])
            rt_s = nrm_pool.tile([P, T], f32, tag="n1q")
            nc.scalar.activation(rt_s[:], q_sA[:], AF.Sqrt, bias=b_tv)
            invt_s = nrm_pool.tile([P, T], f32, tag="n1r")
            nc.vector.reciprocal(invt_s[:], rt_s[:])
            scale_s = nrm_pool.tile([P, T], f32, tag="n1s")
            nc.vector.tensor_mul(scale_s[:], inv_s[:], invt_s[:])

            # ni_raw[t, k] = sum_u sq_v[t, k, u]  (k-major: u innermost)
            ni_raw = nrm_pool.tile([P, T * 3], f32, tag="n1t")
            nc.vector.tensor_reduce(
                ni_raw[:].rearrange("p (t i) -> p t i", i=3),
                sq_v[:].rearrange("p (t k u) -> p t k u", k=3, u=mh),
                axis=AX.X, op=ALU.add)
            iv2 = nrm_pool.tile([P, T], f32, tag="n1u")
            nc.vector.tensor_mul(iv2[:], inv_v[:], inv_v[:])
            ni_tv = nrm_pool.tile([P, T * 3], f32, tag="n1v")
            nc.vector.tensor_mul(
                ni_tv[:].rearrange("p (t i) -> p t i", i=3),
                ni_raw[:].rearrange("p (t i) -> p t i", i=3),
                iv2[:].unsqueeze(2).broadcast_to([P, T, 3]))
            n1tv = nrm_pool.tile([P, T * 3], f32, tag="n1w")
            nc.scalar.activation(n1tv[:], ni_tv[:], AF.Sqrt, bias=b_tv)
            nm3 = nrm_pool.tile([P, T], f32, tag="n1x")
            nc.vector.tensor_reduce(nm3[:], n1tv[:].rearrange("p (t i) -> p t i", i=3),
                                    axis=AX.X, op=ALU.add)
            nme = nrm_pool.tile([P, T], f32, tag="n1y")
            nc.vector.tensor_scalar(nme[:], nm3[:], 1.0 / 3.0, EPS_TV,
                                    op0=ALU.mult, op1=ALU.add)
            invtv = nrm_pool.tile([P, T], f32, tag="n1z")
            nc.vector.reciprocal(invtv[:], nme[:])
            scale_v = nrm_pool.tile([P, T], f32, tag="n1A")
            nc.vector.tensor_mul(scale_v[:], inv_v[:], invtv[:])

            # apply scales -> zmid [128, T*64] (features [s(16) | v u-major xyz])
            zmid = mid_pool.tile([P, T * 64], f32, tag="zmid")
            zmv = zmid[:].rearrange("p (t f) -> p t f", f=64)
            nc.vector.tensor_mul(
                zmv[:, :, 0:MH], ys8v,
                scale_s[:].unsqueeze(2).broadcast_to([P, T, MH]))
            # v: out (u, k) u-major <- yv8 (k, u) k-major, times scale_v
            nc.vector.tensor_mul(
                zmv[:, :, MH:64].rearrange("p t (u k) -> p t u k", k=3, u=mh),
                yv8v.rearrange("p t (k u) -> p t u k", k=3, u=mh),
                scale_v[:].unsqueeze(2).unsqueeze(3).broadcast_to([P, T, mh, 3]))

            # ---- layer 2 ----
            zs8 = nrm_pool.tile([P, T * MO], f32, tag="zs8")
            zv8 = nrm_pool.tile([P, T * 3 * MO], f32, tag="zv8")
            for g in range(T // GRP):
                _emit_layer_group(nc, pools, g, zmid[:], 64, 64, 32, R2 // 128,
                                  u2_sb[:], l2_sb[:], zs8[:], zv8[:],
                                  ident[:], MO)

            # ---- si_norm(2) ----
            mo = MO
            zs8v = zs8[:].rearrange("p (t f) -> p t f", f=mo)
            zv8v = zv8[:].rearrange("p (t f) -> p t f", f=3 * mo)
            sq_s2 = nrm_pool.tile([P, T * mo], f32, tag="sq_s2")
            nc.vector.tensor_mul(sq_s2[:], zs8[:], zs8[:])
            sumsq2 = nrm_pool.tile([P, T], f32, tag="n2a")
            nc.vector.tensor_reduce(sumsq2[:], sq_s2[:].rearrange("p (t f) -> p t f", f=mo),
                                    axis=AX.X, op=ALU.add)
            sum2 = nrm_pool.tile([P, T], f32, tag="n2b")
            nc.vector.tensor_reduce(sum2[:], zs8v, axis=AX.X, op=ALU.add)
            s22 = nrm_pool.tile([P, T], f32, tag="n2c")
            nc.scalar.activation(s22[:], sum2[:], AF.Square, scale=float(mo) ** -0.5)
            varnum2 = nrm_pool.tile([P, T], f32, tag="n2d")
            nc.vector.tensor_sub(varnum2[:], sumsq2[:], s22[:])
            std_s2 = nrm_pool.tile([P, T], f32, tag="n2e")
            nc.scalar.activation(std_s2[:], varnum2[:], AF.Sqrt,
                                 scale=1.0 / (mo - 1), bias=b_tiny)
            stde_s2 = nrm_pool.tile([P, T], f32, tag="n2f")
            nc.vector.tensor_scalar_add(stde_s2[:], std_s2[:], EPS_SI)
            inv_s2 = nrm_pool.tile([P, T], f32, tag="n2g")
            nc.vector.reciprocal(inv_s2[:], stde_s2[:])

            sq_v2 = nrm_pool.tile([P, T * 3 * mo], f32, tag="sq_v2")
            nc.vector.tensor_mul(sq_v2[:], zv8[:], zv8[:])
            n2u2 = nrm_pool.tile([P, T * mo], f32, tag="n2u2")
            nc.vector.tensor_reduce(
                n2u2[:].rearrange("p (t u) -> p t u", u=mo),
                sq_v2[:].rearrange("p (t k u) -> p t u k", k=3, u=mo),
                axis=AX.X, op=ALU.add)
            norm12 = nrm_pool.tile([P, T * mo], f32, tag="norm12")
            nc.scalar.activation(norm12[:], n2u2[:], AF.Sqrt, bias=b_si)
            rn2 = nrm_pool.tile([P, T], f32, tag="n2h")
            nc.vector.tensor_reduce(rn2[:], n2u2[:].rearrange("p (t u) -> p t u", u=mo),
                                    axis=AX.X, op=ALU.add)
            sum_n2 = nrm_pool.tile([P, T], f32, tag="n2i")
            nc.vector.tensor_reduce(sum_n2[:], norm12[:].rearrange("p (t u) -> p t u", u=mo),
                                    axis=AX.X, op=ALU.add)
            s2n2 = nrm_pool.tile([P, T], f32, tag="n2j")
            nc.scalar.activation(s2n2[:], sum_n2[:], AF.Square, scale=float(mo) ** -0.5)
            varn2 = nrm_pool.tile([P, T], f32, tag="n2k")
            nc.vector.tensor_sub(varn2[:], rn2[:], s2n2[:])
            std_v2 = nrm_pool.tile([P, T], f32, tag="n2l")
            nc.scalar.activation(std_v2[:], varn2[:], AF.Sqrt, scale=1.0 / (mo - 1),
                                 bias=b_v2)
            stde_v2 = nrm_pool.tile([P, T], f32, tag="n2m")
            nc.vector.tensor_scalar_add(stde_v2[:], std_v2[:], EPS_SI)
            inv_v2 = nrm_pool.tile([P, T], f32, tag="n2n")
            nc.vector.reciprocal(inv_v2[:], stde_v2[:])

            # scale, sigmoid, assemble output macro [128, T*32]
            outm = io_pool.tile([P, T * 32], f32, tag="outm")
            outv = outm[:].rearrange("p (t f) -> p t f", f=32)
            tmp_s = nrm_pool.tile([P, T * mo], f32, tag="tmp_s")
            nc.vector.tensor_mul(
                tmp_s[:].rearrange("p (t f) -> p t f", f=mo), zs8v,
                inv_s2[:].unsqueeze(2).broadcast_to([P, T, mo]))
            nc.scalar.activation(outv[:, :, 0:MO],
                                 tmp_s[:].rearrange("p (t f) -> p t f", f=mo),
                                 AF.Sigmoid)
            # v out (w, k) w-major <- zv8 (k, w) k-major
            nc.vector.tensor_mul(
                outv[:, :, MO:32].rearrange("p t (w k) -> p t w k", k=3, w=mo),
                zv8v.rearrange("p t (k w) -> p t w k", k=3, w=mo),
                inv_v2[:].unsqueeze(2).unsqueeze(3).broadcast_to([P, T, mo, 3]))

            oview = out_d[m * MACRO:(m + 1) * MACRO, :].rearrange("(t p) f -> p t f", p=P)
            nc.sync.dma_start(out=oview, in_=outm[:].rearrange("p (t f) -> p t f", f=32))


def _build_program(rows, repeat=1):
    import concourse.bacc as bacc
    import concourse.tile as tile
    import concourse.mybir as mybir
    f32 = mybir.dt.float32

    nc = bacc.Bacc("TRN2", target_bir_lowering=False, debug=False,
                   enable_asserts=False, num_devices=NCORES)
    x_d = nc.dram_tensor("x", [rows, 32], f32, kind="ExternalInput").ap()
    u1_d = nc.dram_tensor("u1", [32, R1], f32, kind="ExternalInput").ap()
    l1_d = nc.dram_tensor("l1", [128, (R1 // 128) * 64], f32, kind="ExternalInput").ap()
    u2_d = nc.dram_tensor("u2", [64, R2], f32, kind="ExternalInput").ap()
    l2_d = nc.dram_tensor("l2", [128, (R2 // 128) * 32], f32, kind="ExternalInput").ap()
    out_d = nc.dram_tensor("out", [rows, 32], f32, kind="ExternalOutput").ap()

    with tile.TileContext(nc) as tc:
        _emit_program(nc, tc, x_d, u1_d, l1_d, u2_d, l2_d, out_d, rows, repeat)
    nc.compile()
    return nc


def _get_program(rows, repeat=1):
    key = (rows, repeat)
    if key not in _PROGRAM_CACHE:
        _PROGRAM_CACHE[key] = _build_program(rows, repeat)
    return _PROGRAM_CACHE[key]


_RUN_CACHE = {}


def _get_runner(rows, repeat):
    """Build (once) a cached jitted shard_map executable for the program."""
    key = (rows, repeat)
    if key in _RUN_CACHE:
        return _RUN_CACHE[key]
    import jax
    import numpy as _np
    from jax.sharding import Mesh, PartitionSpec
    try:
        from jax.experimental.shard_map import shard_map
    except Exception:
        from jax.shard_map import shard_map  # newer jax
    from concourse import bass2jax
    import concourse.mybir as mybir

    nc = _get_program(rows, repeat)
    bass2jax.install_neuronx_cc_hook()
    partition_name = nc.partition_id_tensor.name if nc.partition_id_tensor else None
    in_names, out_names, out_avals, zero_outs = [], [], [], []
    for alloc in nc.m.functions[0].allocations:
        if not isinstance(alloc, mybir.MemoryLocationSet):
            continue
        name = alloc.memorylocations[0].name
        if alloc.kind == "ExternalInput":
            if name != partition_name:
                in_names.append(name)
        elif alloc.kind == "ExternalOutput":
            shape = tuple(alloc.tensor_shape)
            dtype = mybir.dt.np(alloc.dtype)
            out_names.append(name)
            out_avals.append(jax.core.ShapedArray(shape, dtype))
            zero_outs.append(_np.zeros(shape, dtype))
    n_params = len(in_names)
    n_outs = len(out_avals)
    all_in_names = list(in_names) + list(out_names)
    if partition_name is not None:
        all_in_names.append(partition_name)
    donate = tuple(range(n_params, n_params + n_outs))

    def _body(*args):
        operands = list(args)
        if partition_name is not None:
            operands.append(bass2jax.partition_id_tensor())
        outs = bass2jax._bass_exec_p.bind(
            *operands,
            out_avals=tuple(out_avals),
            in_names=tuple(all_in_names),
            out_names=tuple(out_names),
            lowering_input_output_aliases=(),
            sim_require_finite=True,
            sim_require_nnan=True,
            nc=nc,
        )
        return tuple(outs)

    devices = jax.devices()[:NCORES]
    mesh = Mesh(_np.asarray(devices), ("core",))
    in_specs = (PartitionSpec("core"),) * (n_params + n_outs)
    out_specs = (PartitionSpec("core"),) * n_outs
    sharded = jax.jit(
        shard_map(_body, mesh=mesh, in_specs=in_specs, out_specs=out_specs,
                  check_rep=False),
        donate_argnums=donate, keep_unused=True,
    )
    runner = (sharded, in_names, out_names, out_avals, zero_outs)
    _RUN_CACHE[key] = runner
    return runner


_ZERO_CACHE = {}


def _run_cached(rows, repeat, full_inputs):
    """full_inputs: dict name -> already-concatenated (NCORES*rows0, ...) array."""
    import numpy as _np
    sharded, in_names, out_names, out_avals, zero_outs = _get_runner(rows, repeat)
    concat_in = [full_inputs[nm] for nm in in_names]
    key = (rows, repeat)
    if key not in _ZERO_CACHE:
        _ZERO_CACHE[key] = [
            _np.zeros((NCORES * z.shape[0], *z.shape[1:]), z.dtype)
            for z in zero_outs]
    out_arrs = sharded(*concat_in, *_ZERO_CACHE[key])
    i = out_names.index("out")
    return _np.asarray(out_arrs[i]).reshape(NCORES, *out_avals[i].shape)


def kernel(x, w1_ss, w1_vv0, w1_sv, w1_vs, w1_vv1,
           w2_ss, w2_vv0, w2_sv, w2_vs, w2_vv1, _trace=False, _repeat=1):
    from concourse import bass_utils

    x = np.asarray(x, dtype=np.float32)
    U1, L1p, U2, L2p = _get_decomp(
        (np.asarray(w1_ss), np.asarray(w1_vv0), np.asarray(w1_sv),
         np.asarray(w1_vs), np.asarray(w1_vv1)),
        (np.asarray(w2_ss), np.asarray(w2_vv0), np.asarray(w2_sv),
         np.asarray(w2_vs), np.asarray(w2_vv1)))

    rows = x.shape[0] // NCORES
    if _trace:
        shards = x.reshape(NCORES, rows, 32)
        in_maps = [{"x": np.ascontiguousarray(shards[i]), "u1": U1, "l1": L1p,
                    "u2": U2, "l2": L2p}
                   for i in range(NCORES)]
        nc = _get_program(rows, _repeat)
        res = bass_utils.run_bass_kernel_spmd(nc, in_maps,
                                              core_ids=list(range(NCORES)),
                                              trace=True)
        out = np.concatenate([res.results[i]["out"] for i in range(NCORES)], axis=0)
        return out, res
    full_inputs = {"x": np.ascontiguousarray(x),
                   "u1": np.tile(U1, (NCORES, 1)),
                   "l1": np.tile(L1p, (NCORES, 1)),
                   "u2": np.tile(U2, (NCORES, 1)),
                   "l2": np.tile(L2p, (NCORES, 1))}
    try:
        per_core = _run_cached(rows, _repeat, full_inputs)
        return np.ascontiguousarray(per_core.reshape(rows * NCORES, 32))
    except Exception:
        shards = x.reshape(NCORES, rows, 32)
        in_maps = [{"x": np.ascontiguousarray(shards[i]), "u1": U1, "l1": L1p,
                    "u2": U2, "l2": L2p}
                   for i in range(NCORES)]
        nc = _get_program(rows, _repeat)
        res = bass_utils.run_bass_kernel_spmd(nc, in_maps,
                                              core_ids=list(range(NCORES)))
        return np.concatenate([res.results[i]["out"] for i in range(NCORES)], axis=0)
